# revision 48
# baseline (speedup 1.0000x reference)
"""Multi-head attention with bias, distributed over 8 trn2 NeuronCores.

Reference computation (per batch b):
    q = (x @ Wq.T) * depth**-0.5 ; k = y @ Wk.T ; v = y @ Wv.T     (per-head split)
    out = softmax(q @ k.T + bias) @ v @ Wo.T

Sharding: 8 cores = 4 batches x 2 head-halves (tensor parallel over heads).
Core c handles batch b = c//2 and heads (c%2)*8 .. +8, all 2048 queries.
Wq/Wk/Wv column-split, Wo row-split; each core emits a PARTIAL output
[D, S] f32 and the host sums the two halves per batch (the "all-reduce").
vs. a query-split this removes the redundant k/v projections (~60us of PE
per core).

Everything bf16 on the PE (fp8 was tried for the attnv with DoubleRow -
2x faster on the PE - but weight quantization error does NOT average down
through softmax: attn and its error shrink by the same sqrt(n_eff), so
fp8 costs ~4-5% rel err, over the 2e-2 budget).

Device-side layout (feature dim on partitions):
    qT/kT [512, 2048] bf16; logits via K=64 quadrant matmuls (2 heads of a
    128-partition tile run concurrently) into f32 PSUM [128 kk, 2x512 q].
    exp: ACT reads PSUM, scale=1/8 folded in, bf16 out; the exp(bias)
    multiply is one in-place DVE/Pool op per slot (stride-0 head
    broadcast, 2-byte operands keep the DVE 2x fast path; a fraction of
    slots go to the otherwise-idle Pool engine).
    attnT_h(+denom row 64) = [v_h | ones].T @ expw   (K=128, M=65)
    normalize: pattn -> sau bf16, denom row out via DMA, one reciprocal
    for both heads, DMA partition-broadcast back, 2x DVE multiply.
    qh0 normalized rows bounce through DRAM (audram) into an[] during
    qh1 (SBUF is fully booked before the x/y/weight pools close).
    out: Wo.T-projection of an chunks, f32, DMAd as produced; the qh0
    half is thunked into qh1's main loop (PE slack), only qh1 drains at
    the tail.

Main loop: 8 iterations = 2 query-halves x 4 head-pairs.  ScalarE exp
(33.5M elems/core at ~124 G elem/s = 270us) is the pacing engine; PE
slack carries the projection thunks; input loads striped over the
sync+gpsimd DMA queues in consumption order, a few early tiles on
scalar.
"""

import numpy as np
import ml_dtypes
from contextlib import ExitStack

import concourse.bass as bass
import concourse.mybir as mybir
import concourse.tile as tile
from concourse import bacc
from concourse.bass_utils import run_bass_kernel_spmd

# full-problem dims (hardcoded per spec)
B, S, D, H = 4, 2048, 1024, 16
DEPTH = D // H            # 64
P = 128
NCORES = 8

HL = H // 2               # 8 local heads per core
DH = HL * DEPTH           # 512 local head dim
NT = D // P               # 8 d_in tiles
WT = DH // P              # 4 weight-out tiles
HT = WT                   # 4 head-pair tiles
KT = S // P               # 16 kv chunks
SL = S // 2               # 1024 q cols per iteration

BF = mybir.dt.bfloat16
F32 = mybir.dt.float32
EXP = mybir.ActivationFunctionType.Exp

TRACE = False
DEBUG = False
last_exec_time_ns = None
last_results = None

# Pool/DVE share SBUF ports: offloading eb-muls to Pool slowed BOTH
# (DVE 650->918ns, Pool 2169ns) and each 2.2us Pool mul stalled the
# in-order PE stream at its attnv.  All eb-muls stay on DVE.
POOL_SLOTS = frozenset()


def _chunks(total, step):
    return [(n0, min(n0 + step, total)) for n0 in range(0, total, step)]


def _attn_body(ctx, tc, io):
    nc = tc.nc
    xT, yT, ebT, wqT, wkT, wvT, woT, outT = (
        io[k] for k in ("xT", "yT", "ebT", "wqT", "wkT", "wvT", "woT",
                        "outT"))

    # ---- persistent pools ----
    ebpool = ctx.enter_context(tc.tile_pool(name="ebpool", bufs=KT))
    qpool = ctx.enter_context(tc.tile_pool(name="qpool", bufs=HT))
    kpool = ctx.enter_context(tc.tile_pool(name="kpool", bufs=HT))
    vpool = ctx.enter_context(tc.tile_pool(name="vpool", bufs=KT))
    epool = ctx.enter_context(tc.tile_pool(name="epool", bufs=6))
    stpool = ctx.enter_context(tc.tile_pool(name="stpool", bufs=2))
    smpool = ctx.enter_context(tc.tile_pool(name="smpool", bufs=2))
    plp = ctx.enter_context(tc.tile_pool(name="plp", bufs=2, space="PSUM"))
    pap = ctx.enter_context(tc.tile_pool(name="pap", bufs=2, space="PSUM"))
    dpool = ctx.enter_context(tc.tile_pool(name="dpool", bufs=1, space="DRAM"))

    q_sb = [qpool.tile([P, S], BF, tag="qT", name=f"q{t}", bufs=HT)
            for t in range(HT)]
    k_sb = [kpool.tile([P, S], BF, tag="kT", name=f"k{t}", bufs=HT)
            for t in range(HT)]
    v_sb = [vpool.tile([P, HL, 66], BF, tag="v66", name=f"v{c}", bufs=KT)
            for c in range(KT)]
    eb_sb = [ebpool.tile([P, SL], BF, tag="eb", name=f"eb{c}", bufs=KT)
             for c in range(KT)]            # qh0 tiles; qh1 pool comes later
    rscr = dpool.tile([2 * H, SL], BF, tag="rscr", name="rscr", bufs=1)
    audram = dpool.tile([DH, SL], BF, tag="audram", name="audram", bufs=1)

    state = {"slot": 0, "eb1": None, "an": None, "wo": None}

    with tc.tile_pool(name="xpool", bufs=NT) as xpool, \
         tc.tile_pool(name="wqpool", bufs=NT) as wqpool:
      with tc.tile_pool(name="ypool", bufs=NT) as ypool, \
           tc.tile_pool(name="wkpool", bufs=NT) as wkpool, \
           tc.tile_pool(name="wvpool", bufs=NT) as wvpool:
        x_sb = [xpool.tile([P, S], BF, tag="xT", name=f"x{t}", bufs=NT)
                for t in range(NT)]
        y_sb = [ypool.tile([P, S], BF, tag="yT", name=f"y{t}", bufs=NT)
                for t in range(NT)]
        wq_sb = [wqpool.tile([P, DH], BF, tag="wq", name=f"wq{t}", bufs=NT)
                 for t in range(NT)]
        wk_sb = [wkpool.tile([P, DH], BF, tag="wk", name=f"wk{t}", bufs=NT)
                 for t in range(NT)]
        wv_sb = [wvpool.tile([P, DH], BF, tag="wv", name=f"wv{t}", bufs=NT)
                 for t in range(NT)]

        # ---- input loads: consumption-priority order, striped over the
        # sync+gpsimd queues; first x tiles on scalar (drains early).
        _q = [0]
        _queues = (nc.sync, nc.gpsimd)

        def dq(out, in_):
            _queues[_q[0] % 2].dma_start(out=out, in_=in_)
            _q[0] += 1

        # first-exp critical set first: q cols 0:512 needs x[:,0:512]+wq
        # block; logits c0-3 need wk block + y[:,0:512]; eb c0 for the mul
        for t in range(NT):
            dq(wq_sb[t][:, 0:P], wqT[t * P:(t + 1) * P, 0:P])
        for t in range(NT):
            if t < 3:
                nc.scalar.dma_start(out=x_sb[t][:, 0:512],
                                    in_=xT[t * P:(t + 1) * P, 0:512])
            else:
                dq(x_sb[t][:, 0:512], xT[t * P:(t + 1) * P, 0:512])
        for t in range(NT):
            dq(wk_sb[t][:, 0:P], wkT[t * P:(t + 1) * P, 0:P])
        for t in range(NT):
            dq(y_sb[t][:, 0:512], yT[t * P:(t + 1) * P, 0:512])
        for c in range(4):
            dq(eb_sb[c], ebT[c * P:(c + 1) * P, 0:SL])
        for t in range(NT):
            dq(x_sb[t][:, 512:1024], xT[t * P:(t + 1) * P, 512:1024])
        for t in range(NT):
            dq(wv_sb[t], wvT[t * P:(t + 1) * P, :])
        for t in range(NT):
            dq(y_sb[t][:, 512:1024], yT[t * P:(t + 1) * P, 512:1024])
        for c in range(4, 8):
            dq(eb_sb[c], ebT[c * P:(c + 1) * P, 0:SL])
        for t in range(NT):
            dq(y_sb[t][:, 1024:2048], yT[t * P:(t + 1) * P, 1024:2048])
        for c in range(8, KT):
            dq(eb_sb[c], ebT[c * P:(c + 1) * P, 0:SL])
        for t in range(NT):
            dq(x_sb[t][:, SL:S], xT[t * P:(t + 1) * P, SL:S])
        for t in range(NT):
            dq(wq_sb[t][:, P:DH], wqT[t * P:(t + 1) * P, P:DH])
        for t in range(NT):
            dq(wk_sb[t][:, P:DH], wkT[t * P:(t + 1) * P, P:DH])

        # ---- warm-up heartbeats (HAM p-state ramp), chained to wq then x
        jnk0 = plp.tile([P, 1024], F32, tag="pl", name="jnk0", bufs=2)
        for t in range(NT):
            nc.tensor.matmul(jnk0[0:1, 0:128], lhsT=wq_sb[t][0:1, 0:1],
                             rhs=wq_sb[t][0:1, 0:128], start=True, stop=True)
        for t in range(NT):
            nc.tensor.matmul(jnk0[0:1, 0:512], lhsT=x_sb[t][0:1, 0:1],
                             rhs=x_sb[t][0:1, 0:512], start=True, stop=True)
            if t == 3:
                for _ in range(10):
                    nc.tensor.matmul(jnk0[0:1, 0:512],
                                     lhsT=x_sb[3][0:1, 0:1],
                                     rhs=x_sb[3][0:1, 0:512],
                                     start=True, stop=True)

        # ---- emission helpers ----
        def emit_q_group(t, n0, n1):
            ps = plp.tile([P, 1024], F32, tag="pl", name=f"psq{t}_{n0}",
                          bufs=2)
            for u in range(NT):
                nc.tensor.matmul(ps[:, 0:n1 - n0],
                                 lhsT=wq_sb[u][:, t * P:(t + 1) * P],
                                 rhs=x_sb[u][:, n0:n1],
                                 start=(u == 0), stop=(u == NT - 1))
            nc.vector.tensor_copy(q_sb[t][:, n0:n1], ps[:, 0:n1 - n0])

        def emit_k_group(t, n0, n1):
            ps = plp.tile([P, 1024], F32, tag="pl", name=f"psk{t}_{n0}",
                          bufs=2)
            for u in range(NT):
                nc.tensor.matmul(ps[:, 0:n1 - n0],
                                 lhsT=wk_sb[u][:, t * P:(t + 1) * P],
                                 rhs=y_sb[u][:, n0:n1],
                                 start=(u == 0), stop=(u == NT - 1))
            nc.vector.tensor_copy(k_sb[t][:, n0:n1], ps[:, 0:n1 - n0])

        def emit_v_group(c, g):
            # kv chunk c, d_out group g (256 wide = 4 heads)
            vt = v_sb[c]
            if g == 0:
                nc.vector.memset(vt[:, :, 64:65], 1.0)
                nc.vector.memset(vt[:, :, 65:66], 0.0)
            n0, n1 = g * 256, (g + 1) * 256
            ps = plp.tile([P, 1024], F32, tag="pl", name=f"psv{c}_{g}",
                          bufs=2)
            for u in range(NT):
                nc.tensor.matmul(ps[:, 0:256],
                                 lhsT=y_sb[u][:, c * P:(c + 1) * P],
                                 rhs=wv_sb[u][:, n0:n1],
                                 start=(u == 0), stop=(u == NT - 1))
            src = ps[:, 0:256].rearrange("p (h d) -> p h d", d=DEPTH)
            nc.vector.tensor_copy(vt[:, 4 * g:4 * g + 4, 0:DEPTH], src)

        # 256-wide thunk units (~0.9us) stay under the ~1.07us exp slot
        # period so released PE bursts never starve the ACT stream
        def q_thunks(t, n0=0, n1=S):
            return [lambda a=a, b=b: emit_q_group(t, a, b)
                    for a, b in _chunks(n1 - n0, 256)
                    for a, b in [(a + n0, b + n0)]]

        def k_thunks(t, n0=0, n1=S):
            return [lambda a=a, b=b: emit_k_group(t, a, b)
                    for a, b in _chunks(n1 - n0, 256)
                    for a, b in [(a + n0, b + n0)]]

        def v_thunks(c):
            return [lambda g=g: emit_v_group(c, g) for g in range(2)]

        # ---- prologue: only the two tiles the first exp fronts need.
        # The q[512:1024] chunk and the v-lead (late wv DMA) are emitted
        # AFTER the pre-phase fronts via post_pre, so the first logits
        # gate on just x/y/w first-chunks (~16us) instead of wv (~46us).
        emit_q_group(0, 0, 512)
        emit_k_group(0, 0, 512)

        def post_pre0():
            emit_q_group(0, 512, 1024)
            for c in range(2):
                for th in v_thunks(c):
                    th()

        # ---- the 8 main iterations ----
        def emit_iter(qh, hp, extra_thunks=(), pre=0, fast_tail=False,
                      pre_thunks=(), an_direct=False, post_pre=None):
            q0 = qh * SL
            ha, hb = 2 * hp, 2 * hp + 1
            eb_cur = eb_sb if qh == 0 else state["eb1"]
            # previous iteration's sau copies: they gate the pattn pool
            # rotation, so they must be emitted before this allocation
            for th in pre_thunks:
                th()
            pattn = [pap.tile([65, SL], F32, tag="pattn",
                              name=f"pa{qh}_{ha + hf}", bufs=2)
                     for hf in range(2)]
            thunks = list(extra_thunks)
            sched = {}
            nsc = max(1, KT - 3)
            for i, th in enumerate(thunks):
                sched.setdefault(i * nsc // max(1, len(thunks)), []).append(th)

            def slot_front(c, n0, n1):
                w = n1 - n0
                plt = plp.tile([P, 1024], F32, tag="pl",
                               name=f"pl{qh}_{hp}_{c}_{n0}", bufs=2)
                nc.tensor.matmul(plt[:, 0:w],
                                 lhsT=k_sb[hp][0:DEPTH, c * P:(c + 1) * P],
                                 rhs=q_sb[hp][0:DEPTH, q0 + n0:q0 + n1],
                                 start=True, stop=True)
                nc.tensor.matmul(plt[:, w:2 * w],
                                 lhsT=k_sb[hp][DEPTH:2 * DEPTH,
                                               c * P:(c + 1) * P],
                                 rhs=q_sb[hp][DEPTH:2 * DEPTH,
                                              q0 + n0:q0 + n1],
                                 start=True, stop=True)
                ew = epool.tile([P, 1024], BF, tag="ew",
                                name=f"ew{hp}_{c}_{n0}", bufs=6)
                # depth**-0.5 folded into the ACT scale port
                nc.scalar.activation(ew[:, 0:2 * w], plt[:, 0:2 * w],
                                     EXP, scale=DEPTH ** -0.5)
                # in-place eb multiply, both heads in one op (stride-0
                # broadcast); a fraction of slots go to the idle Pool
                e3 = ew[:, 0:2 * w].rearrange("p (h w) -> p h w", w=w)
                ebb = (eb_cur[c][:, n0:n1].unsqueeze(1)
                       .broadcast_to([P, 2, w]))
                eng = (nc.gpsimd if (state["slot"] % 16) in POOL_SLOTS
                       else nc.vector)
                eng.tensor_mul(e3, e3, ebb)
                state["slot"] += 1
                return ew

            def emit_attnv(c, n0, n1, ew):
                w = n1 - n0
                nc.tensor.matmul(pattn[0][:, n0:n1],
                                 lhsT=v_sb[c][:, ha, 0:65],
                                 rhs=ew[:, 0:w],
                                 start=(c == 0), stop=(c == KT - 1))
                nc.tensor.matmul(pattn[1][:, n0:n1],
                                 lhsT=v_sb[c][:, hb, 0:65],
                                 rhs=ew[:, w:2 * w],
                                 start=(c == 0), stop=(c == KT - 1))

            # pre-phase: run exp fronts ahead so ACT starts before v ready.
            # n0=0 only - the first fronts then gate on just q[:, 0:512],
            # which arrives ~15us before the full q tile.
            pre_ew = {}
            for c in range(pre):
                pre_ew[(c, 0)] = slot_front(c, 0, 512)
            if post_pre is not None:
                post_pre()

            slotq = []
            pend = []
            for c in range(KT):
                pend.extend(sched.get(c, ()))
                left = max(1, 2 * (KT - 2 - c))
                budget = (max(2, (len(pend) + left - 1) // left)
                          if c < KT - 2 else len(pend))
                for n0, n1 in _chunks(SL, 512):
                    ew = pre_ew.pop((c, n0), None)
                    if ew is None:
                        ew = slot_front(c, n0, n1)
                    slotq.append((c, n0, n1, ew))
                    if len(slotq) > 2:
                        emit_attnv(*slotq.pop(0))
                    for th in pend[:budget]:
                        th()
                    del pend[:budget]
            for th in pend:
                th()
            for args in slotq:
                emit_attnv(*args)

            # ---- normalize: sau bf16 copy, denominator row to partitions
            # 0:2 via DMA, one reciprocal for both heads, partition
            # broadcast, 2x multiply.  qh0 rows bounce via audram.
            # Emitted as THUNKS released early in the NEXT iteration's
            # c-loop, so this ~6.5us DVE burst does not sit in front of
            # the next iteration's eb-muls in the DVE queue (sau copies
            # first - they gate the pattn pool rotation).
            den_t = smpool.tile([2, SL], BF, tag="dent", name=f"dn{qh}{hp}",
                                bufs=1)
            saus = [stpool.tile([65, SL], BF, tag="sau",
                                name=f"sa{qh}_{ha + hf}", bufs=2)
                    for hf in range(2)]
            row = qh * H + 2 * hp

            def sau_copy(hf):
                nc.vector.tensor_copy(saus[hf], pattn[hf])
                nc.sync.dma_start(out=den_t[hf:hf + 1, :],
                                  in_=saus[hf][64:65, :])

            def recip_chain():
                denf = smpool.tile([2, SL], F32, tag="denf",
                                   name=f"df{qh}{hp}", bufs=1)
                nc.vector.tensor_copy(denf, den_t)
                recipf = smpool.tile([2, SL], F32, tag="recipf",
                                     name=f"rf{qh}{hp}", bufs=1)
                nc.vector.reciprocal_approx_fast(recipf, denf)
                recipb = smpool.tile([2, SL], BF, tag="recipb",
                                     name=f"rb{qh}{hp}", bufs=1)
                nc.vector.tensor_copy(recipb, recipf)
                nc.sync.dma_start(out=rscr[row:row + 2, :], in_=recipb)

            def bc_mul(hf):
                bc = smpool.tile([DEPTH, SL], BF, tag="bc",
                                 name=f"bc{qh}_{ha + hf}", bufs=2)
                nc.sync.dma_start(
                    out=bc,
                    in_=rscr[row + hf:row + hf + 1, :].partition_broadcast(
                        DEPTH))
                if qh == 0 and not an_direct:
                    anh = smpool.tile([DEPTH, SL], BF, tag="anh",
                                      name=f"ah{ha + hf}", bufs=2)
                    nc.vector.tensor_mul(anh, saus[hf][0:64, :], bc)
                    nc.sync.dma_start(
                        out=audram[hp * P + hf * DEPTH:
                                   hp * P + (hf + 1) * DEPTH, :],
                        in_=anh)
                else:
                    an_sb = state["an"]
                    nc.vector.tensor_mul(
                        an_sb[hp][hf * DEPTH:(hf + 1) * DEPTH, q0:q0 + SL],
                        saus[hf][0:64, :], bc)

            if not fast_tail:
                return [lambda: sau_copy(0), lambda: sau_copy(1),
                        recip_chain, lambda: bc_mul(0), lambda: bc_mul(1)]

            # last iteration: fully on-chip normalize - PE gathers the
            # denominator row and broadcasts the reciprocal, no DRAM
            # round trips on the critical tail.
            ones = smpool.tile([P, DEPTH], BF, tag="ones", name="ones7",
                               bufs=1)
            nc.vector.memset(ones, 1.0)
            for hf in range(2):
                nc.vector.tensor_copy(saus[hf][64:65, :],
                                      pattn[hf][64:65, :])
                nc.vector.tensor_copy(saus[hf][0:64, :], pattn[hf][0:64, :])
            recips = []
            for hf in range(2):
                dps = plp.tile([P, 1024], F32, tag="pl", name=f"dps{hf}",
                               bufs=2)
                for n0, n1 in _chunks(SL, 512):
                    nc.tensor.matmul(dps[0:1, n0:n1],
                                     lhsT=ones[64:65, 0:1],
                                     rhs=saus[hf][64:65, n0:n1],
                                     start=True, stop=True)
                rcf = smpool.tile([2, SL], F32,
                                  tag="recipf" if hf == 0 else "denf",
                                  name=f"rcf7_{hf}", bufs=1)
                nc.vector.reciprocal_approx_fast(rcf[0:1, :], dps[0:1, :])
                rcb = smpool.tile([2, SL], BF,
                                  tag="recipb" if hf == 0 else "dent",
                                  name=f"rcb7_{hf}", bufs=1)
                nc.vector.tensor_copy(rcb[0:1, :], rcf[0:1, :])
                recips.append(rcb)
            an_sb = state["an"]
            for hf in range(2):
                bcp = plp.tile([P, 1024], F32, tag="pl", name=f"bcp{hf}",
                               bufs=2)
                for n0, n1 in _chunks(SL, 512):
                    nc.tensor.matmul(bcp[0:DEPTH, n0:n1],
                                     lhsT=ones[0:1, 0:DEPTH],
                                     rhs=recips[hf][0:1, n0:n1],
                                     start=True, stop=True)
                nc.vector.tensor_mul(
                    an_sb[hp][hf * DEPTH:(hf + 1) * DEPTH, q0:q0 + SL],
                    saus[hf][0:64, :], bcp[0:DEPTH, :])
            return []

        # iterations 0..2 run inside the full pool scope.  q projections
        # emit only the qh0 columns here; the qh1 halves run in it3 when
        # the PE has slack and ACT pacing is established.
        it0_thunks = k_thunks(0, 512, S) + v_thunks(2) + v_thunks(3)
        for c in range(4, KT):
            it0_thunks += v_thunks(c)
        it0_thunks += q_thunks(1, 0, SL) + k_thunks(1)
        nrm = emit_iter(0, 0, extra_thunks=it0_thunks, pre=6,
                        post_pre=post_pre0)
        nrm = emit_iter(0, 1, pre_thunks=nrm[0:2],
                        extra_thunks=(nrm[2:] + q_thunks(2, 0, SL)
                                      + k_thunks(2) + q_thunks(3, 0, SL)))
        nrm = emit_iter(0, 2, pre_thunks=nrm[0:2],
                        extra_thunks=nrm[2:] + k_thunks(3))

      # y/wk/wv closed; it3 runs the deferred qh1-half q projections
      # (x/wq still resident).
      q1thunks = (q_thunks(0, SL, S) + q_thunks(1, SL, S)
                  + q_thunks(2, SL, S) + q_thunks(3, SL, S))
      nrm = emit_iter(0, 3, pre_thunks=nrm[0:2],
                      extra_thunks=nrm[2:] + q1thunks)
      state["nrm"] = nrm

    # x/wq closed: SBUF reused for eb(qh1), Wo, an, o staging.
    eb1pool = ctx.enter_context(tc.tile_pool(name="eb1pool", bufs=KT))
    wopool = ctx.enter_context(tc.tile_pool(name="wopool", bufs=WT))
    anpool = ctx.enter_context(tc.tile_pool(name="anpool", bufs=HT))
    opool = ctx.enter_context(tc.tile_pool(name="opool", bufs=5))
    eb1_sb = [eb1pool.tile([P, SL], BF, tag="eb1", name=f"eb1_{c}", bufs=KT)
              for c in range(KT)]
    wo_sb = [wopool.tile([P, D], BF, tag="wo", name=f"wo{t}", bufs=WT)
             for t in range(WT)]
    an_sb = [anpool.tile([P, S], BF, tag="an", name=f"an{t}", bufs=HT)
             for t in range(HT)]
    for c in range(KT):
        dq(eb1_sb[c], ebT[c * P:(c + 1) * P, SL:S])
    for t in range(WT):
        dq(wo_sb[t], woT[t * P:(t + 1) * P, :])
    # hp0-2's qh0 rows reload now; hp3's audram rows are written by it3's
    # normalize thunks (released inside it4), so its reload is an it4
    # thunk ordered after them.
    for t in range(HT - 1):
        nc.gpsimd.dma_start(out=an_sb[t][:, 0:SL],
                            in_=audram[t * P:(t + 1) * P, :])

    def reload_an3():
        nc.gpsimd.dma_start(out=an_sb[HT - 1][:, 0:SL],
                            in_=audram[(HT - 1) * P:HT * P, :])

    state["eb1"] = eb1_sb
    state["an"] = an_sb

    def emit_o_half(m, q0, cast_eng=None):
        # output rows m*128, query cols q0..q0+512 (one ~1.7us PE burst)
        ps = plp.tile([P, 1024], F32, tag="pl", name=f"pso{m}_{q0}", bufs=2)
        for t in range(WT):
            nc.tensor.matmul(ps[:, 0:512],
                             lhsT=wo_sb[t][:, m * P:(m + 1) * P],
                             rhs=an_sb[t][:, q0:q0 + 512],
                             start=(t == 0), stop=(t == WT - 1))
        osb = opool.tile([P, 512], F32, tag="osb", name=f"o{m}_{q0}",
                         bufs=5)
        if cast_eng is nc.scalar:
            nc.scalar.copy(osb, ps[:, 0:512])
        else:
            nc.vector.tensor_copy(osb, ps[:, 0:512])
        eng = (nc.sync, nc.gpsimd, nc.scalar)[(m + q0 // 512) % 3]
        eng.dma_start(out=outT[m * P:(m + 1) * P, q0:q0 + 512], in_=osb)

    nrm = state["nrm"]
    o0 = [lambda m=m, h=h: emit_o_half(m, h) for m in range(NT)
          for h in (0, 512)]
    nrm = emit_iter(1, 0, pre_thunks=nrm[0:2],
                    extra_thunks=nrm[2:] + [reload_an3] + o0[0:2])
    nrm = emit_iter(1, 1, pre_thunks=nrm[0:2],
                    extra_thunks=nrm[2:] + o0[2:9])
    nrm = emit_iter(1, 2, pre_thunks=nrm[0:2],
                    extra_thunks=nrm[2:] + o0[9:16])
    emit_iter(1, 3, pre_thunks=nrm[0:2], extra_thunks=nrm[2:],
              fast_tail=True)
    for i, (m, h) in enumerate([(m, h) for m in range(NT)
                                for h in (SL, SL + 512)]):
        emit_o_half(m, h, cast_eng=nc.scalar if i % 2 else nc.vector)

    if DEBUG:
        for t in range(HT):
            nc.sync.dma_start(out=io["anD"][t * P:(t + 1) * P, :],
                              in_=an_sb[t])
            nc.sync.dma_start(out=io["qD"][t * P:(t + 1) * P, :],
                              in_=q_sb[t])
            nc.sync.dma_start(out=io["kD"][t * P:(t + 1) * P, :],
                              in_=k_sb[t])
        nc.sync.dma_start(out=io["rD"], in_=rscr)


def build_nc():
    nc = bacc.Bacc("TRN2", target_bir_lowering=False, debug=False)
    io = {
        "xT": nc.dram_tensor("xT", [D, S], BF, kind="ExternalInput").ap(),
        "yT": nc.dram_tensor("yT", [D, S], BF, kind="ExternalInput").ap(),
        "ebT": nc.dram_tensor("ebT", [S, S], BF, kind="ExternalInput").ap(),
        "wqT": nc.dram_tensor("wqT", [D, DH], BF, kind="ExternalInput").ap(),
        "wkT": nc.dram_tensor("wkT", [D, DH], BF, kind="ExternalInput").ap(),
        "wvT": nc.dram_tensor("wvT", [D, DH], BF, kind="ExternalInput").ap(),
        "woT": nc.dram_tensor("woT", [DH, D], BF, kind="ExternalInput").ap(),
        "outT": nc.dram_tensor("outT", [D, S], F32,
                               kind="ExternalOutput").ap(),
    }
    if DEBUG:
        io["anD"] = nc.dram_tensor("anD", [DH, S], BF,
                                   kind="ExternalOutput").ap()
        io["qD"] = nc.dram_tensor("qD", [DH, S], BF,
                                  kind="ExternalOutput").ap()
        io["kD"] = nc.dram_tensor("kD", [DH, S], BF,
                                  kind="ExternalOutput").ap()
        io["rD"] = nc.dram_tensor("rD", [2 * H, SL], BF,
                                  kind="ExternalOutput").ap()
    with tile.TileContext(nc) as tc:
        with ExitStack() as ctx:
            _attn_body(ctx, tc, io)
    nc.compile()
    return nc


_NC_CACHE = None


def kernel(x, y, bias, Wq, Wk, Wv, Wo):
    global _NC_CACHE, last_exec_time_ns, last_results
    x = np.asarray(x, np.float32)
    y = np.asarray(y, np.float32)
    bias = np.asarray(bias, np.float32)
    Wq, Wk, Wv, Wo = (np.asarray(w, np.float32) for w in (Wq, Wk, Wv, Wo))
    if _NC_CACHE is None:
        _NC_CACHE = build_nc()
    nc = _NC_CACHE

    bf = ml_dtypes.bfloat16
    ebT = np.ascontiguousarray(
        np.exp(bias[0, 0].astype(np.float32)).T).astype(bf)
    xT_all = [np.ascontiguousarray(x[b].T).astype(bf) for b in range(B)]
    yT_all = [np.ascontiguousarray(y[b].T).astype(bf) for b in range(B)]
    wqT = [np.ascontiguousarray(Wq[h * DH:(h + 1) * DH, :].T).astype(bf)
           for h in range(2)]
    wkT = [np.ascontiguousarray(Wk[h * DH:(h + 1) * DH, :].T).astype(bf)
           for h in range(2)]
    wvT = [np.ascontiguousarray(Wv[h * DH:(h + 1) * DH, :].T).astype(bf)
           for h in range(2)]
    woT = [np.ascontiguousarray(Wo.T[h * DH:(h + 1) * DH, :]).astype(bf)
           for h in range(2)]

    in_maps = []
    for core in range(NCORES):
        b, hh = divmod(core, 2)
        in_maps.append({
            "xT": xT_all[b], "yT": yT_all[b], "ebT": ebT,
            "wqT": wqT[hh], "wkT": wkT[hh], "wvT": wvT[hh],
            "woT": woT[hh],
        })

    res = run_bass_kernel_spmd(nc, in_maps, core_ids=list(range(NCORES)),
                               trace=TRACE)
    last_exec_time_ns = res.exec_time_ns
    last_results = res
    out = np.empty((B, S, D), np.float32)
    for b in range(B):
        tot = res.results[2 * b]["outT"] + res.results[2 * b + 1]["outT"]
        out[b] = tot.T
    return out


# revision 49
# speedup vs baseline: 1.1742x; 1.1742x over previous
"""Multi-head attention with bias, distributed over 8 trn2 NeuronCores.

Reference computation (per batch b):
    q = (x @ Wq.T) * depth**-0.5 ; k = y @ Wk.T ; v = y @ Wv.T     (per-head split)
    out = softmax(q @ k.T + bias) @ v @ Wo.T

Sharding: 8 cores = 4 batches x 2 head-halves (tensor parallel over heads).
Core c handles batch b = c//2 and heads (c%2)*8 .. +8, all 2048 queries.
Wq/Wk/Wv column-split, Wo row-split; each core emits a PARTIAL output
[D, S] f32 and the host sums the two halves per batch (the "all-reduce").
vs. a query-split this removes the redundant k/v projections (~60us of PE
per core).

Everything bf16 on the PE (fp8 was tried for the attnv with DoubleRow -
2x faster on the PE - but weight quantization error does NOT average down
through softmax: attn and its error shrink by the same sqrt(n_eff), so
fp8 costs ~4-5% rel err, over the 2e-2 budget).

Device-side layout (feature dim on partitions):
    qT/kT [512, 2048] bf16; logits via K=64 quadrant matmuls (2 heads of a
    128-partition tile run concurrently) into f32 PSUM [128 kk, 2x512 q].
    exp: ACT reads PSUM, scale=1/8 folded in, bf16 out; the exp(bias)
    multiply is one in-place DVE/Pool op per slot (stride-0 head
    broadcast, 2-byte operands keep the DVE 2x fast path; a fraction of
    slots go to the otherwise-idle Pool engine).
    attnT_h(+denom row 64) = [v_h | ones].T @ expw   (K=128, M=65)
    normalize: pattn -> sau bf16, denom row out via DMA, one reciprocal
    for both heads, DMA partition-broadcast back, 2x DVE multiply.
    qh0 normalized rows bounce through DRAM (audram) into an[] during
    qh1 (SBUF is fully booked before the x/y/weight pools close).
    out: Wo.T-projection of an chunks, f32, DMAd as produced; the qh0
    half is thunked into qh1's main loop (PE slack), only qh1 drains at
    the tail.

Main loop: 8 iterations = 2 query-halves x 4 head-pairs.  ScalarE exp
(33.5M elems/core at ~124 G elem/s = 270us) is the pacing engine; PE
slack carries the projection thunks; input loads striped over the
sync+gpsimd DMA queues in consumption order, a few early tiles on
scalar.
"""

import numpy as np
import ml_dtypes
from contextlib import ExitStack

import concourse.bass as bass
import concourse.mybir as mybir
import concourse.tile as tile
from concourse import bacc
from concourse.bass_utils import run_bass_kernel_spmd

# full-problem dims (hardcoded per spec)
B, S, D, H = 4, 2048, 1024, 16
DEPTH = D // H            # 64
P = 128
NCORES = 8

HL = H // 2               # 8 local heads per core
DH = HL * DEPTH           # 512 local head dim
NT = D // P               # 8 d_in tiles
WT = DH // P              # 4 weight-out tiles
HT = WT                   # 4 head-pair tiles
KT = S // P               # 16 kv chunks
SL = S // 2               # 1024 q cols per iteration

BF = mybir.dt.bfloat16
F32 = mybir.dt.float32
EXP = mybir.ActivationFunctionType.Exp

TRACE = False
DEBUG = False
last_exec_time_ns = None
last_results = None

# Pool/DVE share SBUF ports: offloading eb-muls to Pool slowed BOTH
# (DVE 650->918ns, Pool 2169ns) and each 2.2us Pool mul stalled the
# in-order PE stream at its attnv.  All eb-muls stay on DVE.
POOL_SLOTS = frozenset()


def _chunks(total, step):
    return [(n0, min(n0 + step, total)) for n0 in range(0, total, step)]


def _attn_body(ctx, tc, io):
    nc = tc.nc
    xT, yT, ebT, wqT, wkT, wvT, woT, outT = (
        io[k] for k in ("xT", "yT", "ebT", "wqT", "wkT", "wvT", "woT",
                        "outT"))

    # ---- persistent pools ----
    ebpool = ctx.enter_context(tc.tile_pool(name="ebpool", bufs=KT))
    qpool = ctx.enter_context(tc.tile_pool(name="qpool", bufs=HT))
    kpool = ctx.enter_context(tc.tile_pool(name="kpool", bufs=HT))
    vpool = ctx.enter_context(tc.tile_pool(name="vpool", bufs=KT))
    epool = ctx.enter_context(tc.tile_pool(name="epool", bufs=6))
    stpool = ctx.enter_context(tc.tile_pool(name="stpool", bufs=2))
    smpool = ctx.enter_context(tc.tile_pool(name="smpool", bufs=2))
    plp = ctx.enter_context(tc.tile_pool(name="plp", bufs=2, space="PSUM"))
    pap = ctx.enter_context(tc.tile_pool(name="pap", bufs=2, space="PSUM"))
    dpool = ctx.enter_context(tc.tile_pool(name="dpool", bufs=1, space="DRAM"))

    q_sb = [qpool.tile([P, S], BF, tag="qT", name=f"q{t}", bufs=HT)
            for t in range(HT)]
    k_sb = [kpool.tile([P, S], BF, tag="kT", name=f"k{t}", bufs=HT)
            for t in range(HT)]
    v_sb = [vpool.tile([P, HL, 66], BF, tag="v66", name=f"v{c}", bufs=KT)
            for c in range(KT)]
    eb_sb = [ebpool.tile([P, SL], BF, tag="eb", name=f"eb{c}", bufs=KT)
             for c in range(KT)]            # qh0 tiles; qh1 pool comes later
    rscr = dpool.tile([2 * H, SL], BF, tag="rscr", name="rscr", bufs=1)
    audram = dpool.tile([DH, SL], BF, tag="audram", name="audram", bufs=1)

    state = {"slot": 0, "eb1": None, "an": None, "wo": None}

    with tc.tile_pool(name="xpool", bufs=NT) as xpool, \
         tc.tile_pool(name="wqpool", bufs=NT) as wqpool:
      with tc.tile_pool(name="ypool", bufs=NT) as ypool, \
           tc.tile_pool(name="wkpool", bufs=NT) as wkpool, \
           tc.tile_pool(name="wvpool", bufs=NT) as wvpool:
        x_sb = [xpool.tile([P, S], BF, tag="xT", name=f"x{t}", bufs=NT)
                for t in range(NT)]
        y_sb = [ypool.tile([P, S], BF, tag="yT", name=f"y{t}", bufs=NT)
                for t in range(NT)]
        wq_sb = [wqpool.tile([P, DH], BF, tag="wq", name=f"wq{t}", bufs=NT)
                 for t in range(NT)]
        wk_sb = [wkpool.tile([P, DH], BF, tag="wk", name=f"wk{t}", bufs=NT)
                 for t in range(NT)]
        wv_sb = [wvpool.tile([P, DH], BF, tag="wv", name=f"wv{t}", bufs=NT)
                 for t in range(NT)]

        # ---- input loads: consumption-priority order, striped over the
        # sync+gpsimd queues; first x tiles on scalar (drains early).
        _q = [0]
        _queues = (nc.sync, nc.gpsimd)

        def dq(out, in_):
            _queues[_q[0] % 2].dma_start(out=out, in_=in_)
            _q[0] += 1

        # first-exp critical set first: q cols 0:512 needs x[:,0:512]+wq
        # block; logits c0-3 need wk block + y[:,0:512]; eb c0 for the mul
        for t in range(NT):
            dq(wq_sb[t][:, 0:P], wqT[t * P:(t + 1) * P, 0:P])
        for t in range(NT):
            if t < 3:
                nc.scalar.dma_start(out=x_sb[t][:, 0:512],
                                    in_=xT[t * P:(t + 1) * P, 0:512])
            else:
                dq(x_sb[t][:, 0:512], xT[t * P:(t + 1) * P, 0:512])
        for t in range(NT):
            dq(wk_sb[t][:, 0:P], wkT[t * P:(t + 1) * P, 0:P])
        for t in range(NT):
            dq(y_sb[t][:, 0:512], yT[t * P:(t + 1) * P, 0:512])
        for c in range(4):
            dq(eb_sb[c], ebT[c * P:(c + 1) * P, 0:SL])
        for t in range(NT):
            dq(x_sb[t][:, 512:1024], xT[t * P:(t + 1) * P, 512:1024])
        for t in range(NT):
            dq(wv_sb[t], wvT[t * P:(t + 1) * P, :])
        for t in range(NT):
            dq(y_sb[t][:, 512:1024], yT[t * P:(t + 1) * P, 512:1024])
        for c in range(4, 8):
            dq(eb_sb[c], ebT[c * P:(c + 1) * P, 0:SL])
        for t in range(NT):
            dq(y_sb[t][:, 1024:2048], yT[t * P:(t + 1) * P, 1024:2048])
        for c in range(8, KT):
            dq(eb_sb[c], ebT[c * P:(c + 1) * P, 0:SL])
        for t in range(NT):
            dq(x_sb[t][:, SL:S], xT[t * P:(t + 1) * P, SL:S])
        for t in range(NT):
            dq(wq_sb[t][:, P:DH], wqT[t * P:(t + 1) * P, P:DH])
        for t in range(NT):
            dq(wk_sb[t][:, P:DH], wkT[t * P:(t + 1) * P, P:DH])

        # ---- warm-up heartbeats (HAM p-state ramp), chained to wq then x
        jnk0 = plp.tile([P, 1024], F32, tag="pl", name="jnk0", bufs=2)
        for t in range(NT):
            nc.tensor.matmul(jnk0[0:1, 0:128], lhsT=wq_sb[t][0:1, 0:1],
                             rhs=wq_sb[t][0:1, 0:128], start=True, stop=True)
        for t in range(NT):
            nc.tensor.matmul(jnk0[0:1, 0:512], lhsT=x_sb[t][0:1, 0:1],
                             rhs=x_sb[t][0:1, 0:512], start=True, stop=True)
            if t == 3:
                for _ in range(10):
                    nc.tensor.matmul(jnk0[0:1, 0:512],
                                     lhsT=x_sb[3][0:1, 0:1],
                                     rhs=x_sb[3][0:1, 0:512],
                                     start=True, stop=True)

        # ---- emission helpers ----
        def emit_q_group(t, n0, n1):
            ps = plp.tile([P, 1024], F32, tag="pl", name=f"psq{t}_{n0}",
                          bufs=2)
            for u in range(NT):
                nc.tensor.matmul(ps[:, 0:n1 - n0],
                                 lhsT=wq_sb[u][:, t * P:(t + 1) * P],
                                 rhs=x_sb[u][:, n0:n1],
                                 start=(u == 0), stop=(u == NT - 1))
            nc.vector.tensor_copy(q_sb[t][:, n0:n1], ps[:, 0:n1 - n0])

        def emit_k_group(t, n0, n1):
            ps = plp.tile([P, 1024], F32, tag="pl", name=f"psk{t}_{n0}",
                          bufs=2)
            for u in range(NT):
                nc.tensor.matmul(ps[:, 0:n1 - n0],
                                 lhsT=wk_sb[u][:, t * P:(t + 1) * P],
                                 rhs=y_sb[u][:, n0:n1],
                                 start=(u == 0), stop=(u == NT - 1))
            nc.vector.tensor_copy(k_sb[t][:, n0:n1], ps[:, 0:n1 - n0])

        def emit_v_group(c, g):
            # kv chunk c, d_out group g (256 wide = 4 heads)
            vt = v_sb[c]
            if g == 0:
                nc.vector.memset(vt[:, :, 64:65], 1.0)
                nc.vector.memset(vt[:, :, 65:66], 0.0)
            n0, n1 = g * 256, (g + 1) * 256
            ps = plp.tile([P, 1024], F32, tag="pl", name=f"psv{c}_{g}",
                          bufs=2)
            for u in range(NT):
                nc.tensor.matmul(ps[:, 0:256],
                                 lhsT=y_sb[u][:, c * P:(c + 1) * P],
                                 rhs=wv_sb[u][:, n0:n1],
                                 start=(u == 0), stop=(u == NT - 1))
            src = ps[:, 0:256].rearrange("p (h d) -> p h d", d=DEPTH)
            nc.vector.tensor_copy(vt[:, 4 * g:4 * g + 4, 0:DEPTH], src)

        # 256-wide thunk units (~0.9us) stay under the ~1.07us exp slot
        # period so released PE bursts never starve the ACT stream
        def q_thunks(t, n0=0, n1=S):
            return [lambda a=a, b=b: emit_q_group(t, a, b)
                    for a, b in _chunks(n1 - n0, 256)
                    for a, b in [(a + n0, b + n0)]]

        def k_thunks(t, n0=0, n1=S):
            return [lambda a=a, b=b: emit_k_group(t, a, b)
                    for a, b in _chunks(n1 - n0, 256)
                    for a, b in [(a + n0, b + n0)]]

        def v_thunks(c):
            return [lambda g=g: emit_v_group(c, g) for g in range(2)]

        # ---- prologue: only the two tiles the first exp fronts need.
        # The q[512:1024] chunk and the v-lead (late wv DMA) are emitted
        # AFTER the pre-phase fronts via post_pre, so the first logits
        # gate on just x/y/w first-chunks (~16us) instead of wv (~46us).
        emit_q_group(0, 0, 512)
        emit_k_group(0, 0, 512)

        def post_pre0():
            emit_q_group(0, 512, 1024)
            for c in range(2):
                for th in v_thunks(c):
                    th()

        # ---- the 8 main iterations ----
        def emit_iter(qh, hp, extra_thunks=(), pre=0, fast_tail=False,
                      pre_thunks=(), an_direct=False, post_pre=None):
            q0 = qh * SL
            ha, hb = 2 * hp, 2 * hp + 1
            eb_cur = eb_sb if qh == 0 else state["eb1"]
            # previous iteration's sau copies: they gate the pattn pool
            # rotation, so they must be emitted before this allocation
            for th in pre_thunks:
                th()
            pattn = [pap.tile([65, SL], F32, tag="pattn",
                              name=f"pa{qh}_{ha + hf}", bufs=2)
                     for hf in range(2)]
            thunks = list(extra_thunks)
            sched = {}
            nsc = max(1, KT - 3)
            for i, th in enumerate(thunks):
                sched.setdefault(i * nsc // max(1, len(thunks)), []).append(th)

            def slot_front(c, n0, n1):
                w = n1 - n0
                plt = plp.tile([P, 1024], F32, tag="pl",
                               name=f"pl{qh}_{hp}_{c}_{n0}", bufs=2)
                nc.tensor.matmul(plt[:, 0:w],
                                 lhsT=k_sb[hp][0:DEPTH, c * P:(c + 1) * P],
                                 rhs=q_sb[hp][0:DEPTH, q0 + n0:q0 + n1],
                                 start=True, stop=True)
                nc.tensor.matmul(plt[:, w:2 * w],
                                 lhsT=k_sb[hp][DEPTH:2 * DEPTH,
                                               c * P:(c + 1) * P],
                                 rhs=q_sb[hp][DEPTH:2 * DEPTH,
                                              q0 + n0:q0 + n1],
                                 start=True, stop=True)
                ew = epool.tile([P, 1024], BF, tag="ew",
                                name=f"ew{hp}_{c}_{n0}", bufs=6)
                # depth**-0.5 folded into the ACT scale port
                nc.scalar.activation(ew[:, 0:2 * w], plt[:, 0:2 * w],
                                     EXP, scale=DEPTH ** -0.5)
                # in-place eb multiply, both heads in one op (stride-0
                # broadcast); a fraction of slots go to the idle Pool
                e3 = ew[:, 0:2 * w].rearrange("p (h w) -> p h w", w=w)
                ebb = (eb_cur[c][:, n0:n1].unsqueeze(1)
                       .broadcast_to([P, 2, w]))
                eng = (nc.gpsimd if (state["slot"] % 16) in POOL_SLOTS
                       else nc.vector)
                eng.tensor_mul(e3, e3, ebb)
                state["slot"] += 1
                return ew

            def emit_attnv(c, n0, n1, ew):
                w = n1 - n0
                nc.tensor.matmul(pattn[0][:, n0:n1],
                                 lhsT=v_sb[c][:, ha, 0:65],
                                 rhs=ew[:, 0:w],
                                 start=(c == 0), stop=(c == KT - 1))
                nc.tensor.matmul(pattn[1][:, n0:n1],
                                 lhsT=v_sb[c][:, hb, 0:65],
                                 rhs=ew[:, w:2 * w],
                                 start=(c == 0), stop=(c == KT - 1))

            # pre-phase: run exp fronts ahead so ACT starts before v ready.
            # n0=0 only - the first fronts then gate on just q[:, 0:512],
            # which arrives ~15us before the full q tile.
            pre_ew = {}
            for c in range(pre):
                pre_ew[(c, 0)] = slot_front(c, 0, 512)
            if post_pre is not None:
                post_pre()

            slotq = []
            pend = []
            for c in range(KT):
                pend.extend(sched.get(c, ()))
                left = max(1, 2 * (KT - 2 - c))
                budget = (max(2, (len(pend) + left - 1) // left)
                          if c < KT - 2 else len(pend))
                for n0, n1 in _chunks(SL, 512):
                    ew = pre_ew.pop((c, n0), None)
                    if ew is None:
                        ew = slot_front(c, n0, n1)
                    slotq.append((c, n0, n1, ew))
                    if len(slotq) > 2:
                        emit_attnv(*slotq.pop(0))
                    for th in pend[:budget]:
                        th()
                    del pend[:budget]
            for th in pend:
                th()
            for args in slotq:
                emit_attnv(*args)

            # ---- normalize: sau bf16 copy, denominator row to partitions
            # 0:2 via DMA, one reciprocal for both heads, partition
            # broadcast, 2x multiply.  qh0 rows bounce via audram.
            # Emitted as THUNKS released early in the NEXT iteration's
            # c-loop, so this ~6.5us DVE burst does not sit in front of
            # the next iteration's eb-muls in the DVE queue (sau copies
            # first - they gate the pattn pool rotation).
            den_t = smpool.tile([2, SL], BF, tag="dent", name=f"dn{qh}{hp}",
                                bufs=1)
            saus = [stpool.tile([65, SL], BF, tag="sau",
                                name=f"sa{qh}_{ha + hf}", bufs=2)
                    for hf in range(2)]
            row = qh * H + 2 * hp

            def sau_copy(hf):
                nc.vector.tensor_copy(saus[hf], pattn[hf])
                nc.sync.dma_start(out=den_t[hf:hf + 1, :],
                                  in_=saus[hf][64:65, :])

            def recip_chain():
                denf = smpool.tile([2, SL], F32, tag="denf",
                                   name=f"df{qh}{hp}", bufs=1)
                nc.vector.tensor_copy(denf, den_t)
                recipf = smpool.tile([2, SL], F32, tag="recipf",
                                     name=f"rf{qh}{hp}", bufs=1)
                nc.vector.reciprocal_approx_fast(recipf, denf)
                recipb = smpool.tile([2, SL], BF, tag="recipb",
                                     name=f"rb{qh}{hp}", bufs=1)
                nc.vector.tensor_copy(recipb, recipf)
                nc.sync.dma_start(out=rscr[row:row + 2, :], in_=recipb)

            def bc_mul(hf):
                bc = smpool.tile([DEPTH, SL], BF, tag="bc",
                                 name=f"bc{qh}_{ha + hf}", bufs=2)
                nc.sync.dma_start(
                    out=bc,
                    in_=rscr[row + hf:row + hf + 1, :].partition_broadcast(
                        DEPTH))
                if qh == 0 and not an_direct:
                    anh = smpool.tile([DEPTH, SL], BF, tag="anh",
                                      name=f"ah{ha + hf}", bufs=2)
                    nc.vector.tensor_mul(anh, saus[hf][0:64, :], bc)
                    nc.sync.dma_start(
                        out=audram[hp * P + hf * DEPTH:
                                   hp * P + (hf + 1) * DEPTH, :],
                        in_=anh)
                else:
                    an_sb = state["an"]
                    nc.vector.tensor_mul(
                        an_sb[hp][hf * DEPTH:(hf + 1) * DEPTH, q0:q0 + SL],
                        saus[hf][0:64, :], bc)

            if not fast_tail:
                return [lambda: sau_copy(0), lambda: sau_copy(1),
                        recip_chain, lambda: bc_mul(0), lambda: bc_mul(1)]

            # last iteration: fully on-chip normalize - PE gathers the
            # denominator row and broadcasts the reciprocal, no DRAM
            # round trips on the critical tail.
            ones = smpool.tile([P, DEPTH], BF, tag="ones", name="ones7",
                               bufs=1)
            nc.vector.memset(ones, 1.0)
            for hf in range(2):
                nc.vector.tensor_copy(saus[hf][64:65, :],
                                      pattn[hf][64:65, :])
                nc.vector.tensor_copy(saus[hf][0:64, :], pattn[hf][0:64, :])
            recips = []
            for hf in range(2):
                dps = plp.tile([P, 1024], F32, tag="pl", name=f"dps{hf}",
                               bufs=2)
                for n0, n1 in _chunks(SL, 512):
                    nc.tensor.matmul(dps[0:1, n0:n1],
                                     lhsT=ones[64:65, 0:1],
                                     rhs=saus[hf][64:65, n0:n1],
                                     start=True, stop=True)
                rcf = smpool.tile([2, SL], F32,
                                  tag="recipf" if hf == 0 else "denf",
                                  name=f"rcf7_{hf}", bufs=1)
                nc.vector.reciprocal_approx_fast(rcf[0:1, :], dps[0:1, :])
                rcb = smpool.tile([2, SL], BF,
                                  tag="recipb" if hf == 0 else "dent",
                                  name=f"rcb7_{hf}", bufs=1)
                nc.vector.tensor_copy(rcb[0:1, :], rcf[0:1, :])
                recips.append(rcb)
            an_sb = state["an"]
            for hf in range(2):
                bcp = plp.tile([P, 1024], F32, tag="pl", name=f"bcp{hf}",
                               bufs=2)
                for n0, n1 in _chunks(SL, 512):
                    nc.tensor.matmul(bcp[0:DEPTH, n0:n1],
                                     lhsT=ones[0:1, 0:DEPTH],
                                     rhs=recips[hf][0:1, n0:n1],
                                     start=True, stop=True)
                nc.vector.tensor_mul(
                    an_sb[hp][hf * DEPTH:(hf + 1) * DEPTH, q0:q0 + SL],
                    saus[hf][0:64, :], bcp[0:DEPTH, :])
            return []

        # iterations 0..2 run inside the full pool scope.  q projections
        # emit only the qh0 columns here; the qh1 halves run in it3 when
        # the PE has slack and ACT pacing is established.
        it0_thunks = k_thunks(0, 512, S) + v_thunks(2) + v_thunks(3)
        for c in range(4, KT):
            it0_thunks += v_thunks(c)
        it0_thunks += q_thunks(1, 0, SL) + k_thunks(1)
        nrm = emit_iter(0, 0, extra_thunks=it0_thunks, pre=6,
                        post_pre=post_pre0)
        nrm = emit_iter(0, 1, pre_thunks=nrm[0:2],
                        extra_thunks=(nrm[2:] + q_thunks(2, 0, SL)
                                      + k_thunks(2) + q_thunks(3, 0, SL)))
        nrm = emit_iter(0, 2, pre_thunks=nrm[0:2],
                        extra_thunks=nrm[2:] + k_thunks(3))

      # y/wk/wv closed; it3 runs the deferred qh1-half q projections
      # (x/wq still resident).
      q1thunks = (q_thunks(0, SL, S) + q_thunks(1, SL, S)
                  + q_thunks(2, SL, S) + q_thunks(3, SL, S))
      nrm = emit_iter(0, 3, pre_thunks=nrm[0:2],
                      extra_thunks=nrm[2:] + q1thunks)
      state["nrm"] = nrm

    # x/wq closed: SBUF reused for eb(qh1), Wo, an, o staging.
    eb1pool = ctx.enter_context(tc.tile_pool(name="eb1pool", bufs=KT))
    wopool = ctx.enter_context(tc.tile_pool(name="wopool", bufs=WT))
    anpool = ctx.enter_context(tc.tile_pool(name="anpool", bufs=HT))
    opool = ctx.enter_context(tc.tile_pool(name="opool", bufs=5))
    eb1_sb = [eb1pool.tile([P, SL], BF, tag="eb1", name=f"eb1_{c}", bufs=KT)
              for c in range(KT)]
    wo_sb = [wopool.tile([P, D], BF, tag="wo", name=f"wo{t}", bufs=WT)
             for t in range(WT)]
    an_sb = [anpool.tile([P, S], BF, tag="an", name=f"an{t}", bufs=HT)
             for t in range(HT)]
    for c in range(KT):
        dq(eb1_sb[c], ebT[c * P:(c + 1) * P, SL:S])
    for t in range(WT):
        dq(wo_sb[t], woT[t * P:(t + 1) * P, :])
    # hp0-2's qh0 rows reload now; hp3's audram rows are written by it3's
    # normalize thunks (released inside it4), so its reload is an it4
    # thunk ordered after them.
    for t in range(HT - 1):
        nc.gpsimd.dma_start(out=an_sb[t][:, 0:SL],
                            in_=audram[t * P:(t + 1) * P, :])

    def reload_an3():
        nc.gpsimd.dma_start(out=an_sb[HT - 1][:, 0:SL],
                            in_=audram[(HT - 1) * P:HT * P, :])

    state["eb1"] = eb1_sb
    state["an"] = an_sb

    def emit_o_half(m, q0, cast_eng=None):
        # output rows m*128, query cols q0..q0+512 (one ~1.7us PE burst)
        ps = plp.tile([P, 1024], F32, tag="pl", name=f"pso{m}_{q0}", bufs=2)
        for t in range(WT):
            nc.tensor.matmul(ps[:, 0:512],
                             lhsT=wo_sb[t][:, m * P:(m + 1) * P],
                             rhs=an_sb[t][:, q0:q0 + 512],
                             start=(t == 0), stop=(t == WT - 1))
        osb = opool.tile([P, 512], F32, tag="osb", name=f"o{m}_{q0}",
                         bufs=5)
        if cast_eng is nc.scalar:
            nc.scalar.copy(osb, ps[:, 0:512])
        else:
            nc.vector.tensor_copy(osb, ps[:, 0:512])
        eng = (nc.sync, nc.gpsimd, nc.scalar)[(m + q0 // 512) % 3]
        eng.dma_start(out=outT[m * P:(m + 1) * P, q0:q0 + 512], in_=osb)

    nrm = state["nrm"]
    o0 = [lambda m=m, h=h: emit_o_half(m, h) for m in range(NT)
          for h in (0, 512)]
    # qh0 o-halves cast on ACT: it has structural gaps mid-stream while
    # the DVE queue is the muls' critical path
    o0s = [lambda m=m, h=h: emit_o_half(m, h, cast_eng=nc.scalar)
           for m in range(NT) for h in (0, 512)]
    nrm = emit_iter(1, 0, pre_thunks=nrm[0:2],
                    extra_thunks=nrm[2:] + [reload_an3] + o0s[0:2])
    nrm = emit_iter(1, 1, pre_thunks=nrm[0:2],
                    extra_thunks=nrm[2:] + o0s[2:9])
    nrm = emit_iter(1, 2, pre_thunks=nrm[0:2],
                    extra_thunks=nrm[2:] + o0s[9:16])
    emit_iter(1, 3, pre_thunks=nrm[0:2], extra_thunks=nrm[2:],
              fast_tail=True)
    for i, (m, h) in enumerate([(m, h) for m in range(NT)
                                for h in (SL, SL + 512)]):
        emit_o_half(m, h, cast_eng=nc.scalar if i % 2 else nc.vector)

    if DEBUG:
        for t in range(HT):
            nc.sync.dma_start(out=io["anD"][t * P:(t + 1) * P, :],
                              in_=an_sb[t])
            nc.sync.dma_start(out=io["qD"][t * P:(t + 1) * P, :],
                              in_=q_sb[t])
            nc.sync.dma_start(out=io["kD"][t * P:(t + 1) * P, :],
                              in_=k_sb[t])
        nc.sync.dma_start(out=io["rD"], in_=rscr)


def build_nc():
    nc = bacc.Bacc("TRN2", target_bir_lowering=False, debug=False)
    io = {
        "xT": nc.dram_tensor("xT", [D, S], BF, kind="ExternalInput").ap(),
        "yT": nc.dram_tensor("yT", [D, S], BF, kind="ExternalInput").ap(),
        "ebT": nc.dram_tensor("ebT", [S, S], BF, kind="ExternalInput").ap(),
        "wqT": nc.dram_tensor("wqT", [D, DH], BF, kind="ExternalInput").ap(),
        "wkT": nc.dram_tensor("wkT", [D, DH], BF, kind="ExternalInput").ap(),
        "wvT": nc.dram_tensor("wvT", [D, DH], BF, kind="ExternalInput").ap(),
        "woT": nc.dram_tensor("woT", [DH, D], BF, kind="ExternalInput").ap(),
        "outT": nc.dram_tensor("outT", [D, S], F32,
                               kind="ExternalOutput").ap(),
    }
    if DEBUG:
        io["anD"] = nc.dram_tensor("anD", [DH, S], BF,
                                   kind="ExternalOutput").ap()
        io["qD"] = nc.dram_tensor("qD", [DH, S], BF,
                                  kind="ExternalOutput").ap()
        io["kD"] = nc.dram_tensor("kD", [DH, S], BF,
                                  kind="ExternalOutput").ap()
        io["rD"] = nc.dram_tensor("rD", [2 * H, SL], BF,
                                  kind="ExternalOutput").ap()
    with tile.TileContext(nc) as tc:
        with ExitStack() as ctx:
            _attn_body(ctx, tc, io)
    nc.compile()
    return nc


_NC_CACHE = None


def kernel(x, y, bias, Wq, Wk, Wv, Wo):
    global _NC_CACHE, last_exec_time_ns, last_results
    x = np.asarray(x, np.float32)
    y = np.asarray(y, np.float32)
    bias = np.asarray(bias, np.float32)
    Wq, Wk, Wv, Wo = (np.asarray(w, np.float32) for w in (Wq, Wk, Wv, Wo))
    if _NC_CACHE is None:
        _NC_CACHE = build_nc()
    nc = _NC_CACHE

    bf = ml_dtypes.bfloat16
    ebT = np.ascontiguousarray(
        np.exp(bias[0, 0].astype(np.float32)).T).astype(bf)
    xT_all = [np.ascontiguousarray(x[b].T).astype(bf) for b in range(B)]
    yT_all = [np.ascontiguousarray(y[b].T).astype(bf) for b in range(B)]
    wqT = [np.ascontiguousarray(Wq[h * DH:(h + 1) * DH, :].T).astype(bf)
           for h in range(2)]
    wkT = [np.ascontiguousarray(Wk[h * DH:(h + 1) * DH, :].T).astype(bf)
           for h in range(2)]
    wvT = [np.ascontiguousarray(Wv[h * DH:(h + 1) * DH, :].T).astype(bf)
           for h in range(2)]
    woT = [np.ascontiguousarray(Wo.T[h * DH:(h + 1) * DH, :]).astype(bf)
           for h in range(2)]

    in_maps = []
    for core in range(NCORES):
        b, hh = divmod(core, 2)
        in_maps.append({
            "xT": xT_all[b], "yT": yT_all[b], "ebT": ebT,
            "wqT": wqT[hh], "wkT": wkT[hh], "wvT": wvT[hh],
            "woT": woT[hh],
        })

    res = run_bass_kernel_spmd(nc, in_maps, core_ids=list(range(NCORES)),
                               trace=TRACE)
    last_exec_time_ns = res.exec_time_ns
    last_results = res
    out = np.empty((B, S, D), np.float32)
    for b in range(B):
        tot = res.results[2 * b]["outT"] + res.results[2 * b + 1]["outT"]
        out[b] = tot.T
    return out


# revision 53
# speedup vs baseline: 1.1842x; 1.0084x over previous
"""Multi-head attention with bias, distributed over 8 trn2 NeuronCores.

Reference computation (per batch b):
    q = (x @ Wq.T) * depth**-0.5 ; k = y @ Wk.T ; v = y @ Wv.T     (per-head split)
    out = softmax(q @ k.T + bias) @ v @ Wo.T

Sharding: 8 cores = 4 batches x 2 head-halves (tensor parallel over heads).
Core c handles batch b = c//2 and heads (c%2)*8 .. +8, all 2048 queries.
Wq/Wk/Wv column-split, Wo row-split; each core emits a PARTIAL output
[D, S] f32 and the host sums the two halves per batch (the "all-reduce").
vs. a query-split this removes the redundant k/v projections (~60us of PE
per core).

Everything bf16 on the PE (fp8 was tried for the attnv with DoubleRow -
2x faster on the PE - but weight quantization error does NOT average down
through softmax: attn and its error shrink by the same sqrt(n_eff), so
fp8 costs ~4-5% rel err, over the 2e-2 budget).

Device-side layout (feature dim on partitions):
    qT/kT [512, 2048] bf16; logits via K=64 quadrant matmuls (2 heads of a
    128-partition tile run concurrently) into f32 PSUM [128 kk, 2x512 q].
    exp: ACT reads PSUM, scale=1/8 folded in, bf16 out; the exp(bias)
    multiply is one in-place DVE/Pool op per slot (stride-0 head
    broadcast, 2-byte operands keep the DVE 2x fast path; a fraction of
    slots go to the otherwise-idle Pool engine).
    attnT_h(+denom row 64) = [v_h | ones].T @ expw   (K=128, M=65)
    normalize: pattn -> sau bf16, denom row out via DMA, one reciprocal
    for both heads, DMA partition-broadcast back, 2x DVE multiply.
    qh0 normalized rows bounce through DRAM (audram) into an[] during
    qh1 (SBUF is fully booked before the x/y/weight pools close).
    out: Wo.T-projection of an chunks, f32, DMAd as produced; the qh0
    half is thunked into qh1's main loop (PE slack), only qh1 drains at
    the tail.

Main loop: 8 iterations = 2 query-halves x 4 head-pairs.  ScalarE exp
(33.5M elems/core at ~124 G elem/s = 270us) is the pacing engine; PE
slack carries the projection thunks; input loads striped over the
sync+gpsimd DMA queues in consumption order, a few early tiles on
scalar.
"""

import numpy as np
import ml_dtypes
from contextlib import ExitStack

import concourse.bass as bass
import concourse.mybir as mybir
import concourse.tile as tile
from concourse import bacc
from concourse.bass_utils import run_bass_kernel_spmd

# full-problem dims (hardcoded per spec)
B, S, D, H = 4, 2048, 1024, 16
DEPTH = D // H            # 64
P = 128
NCORES = 8

HL = H // 2               # 8 local heads per core
DH = HL * DEPTH           # 512 local head dim
NT = D // P               # 8 d_in tiles
WT = DH // P              # 4 weight-out tiles
HT = WT                   # 4 head-pair tiles
KT = S // P               # 16 kv chunks
SL = S // 2               # 1024 q cols per iteration

BF = mybir.dt.bfloat16
F32 = mybir.dt.float32
EXP = mybir.ActivationFunctionType.Exp

TRACE = False
DEBUG = False
last_exec_time_ns = None
last_results = None

# Pool/DVE share SBUF ports: offloading eb-muls to Pool slowed BOTH
# (DVE 650->918ns, Pool 2169ns) and each 2.2us Pool mul stalled the
# in-order PE stream at its attnv.  All eb-muls stay on DVE.
POOL_SLOTS = frozenset()


def _chunks(total, step):
    return [(n0, min(n0 + step, total)) for n0 in range(0, total, step)]


def _attn_body(ctx, tc, io):
    nc = tc.nc
    xT, yT, ebT, wqT, wkT, wvT, woT, outT = (
        io[k] for k in ("xT", "yT", "ebT", "wqT", "wkT", "wvT", "woT",
                        "outT"))

    # ---- persistent pools ----
    ebpool = ctx.enter_context(tc.tile_pool(name="ebpool", bufs=KT))
    qpool = ctx.enter_context(tc.tile_pool(name="qpool", bufs=HT))
    kpool = ctx.enter_context(tc.tile_pool(name="kpool", bufs=HT))
    vpool = ctx.enter_context(tc.tile_pool(name="vpool", bufs=KT))
    epool = ctx.enter_context(tc.tile_pool(name="epool", bufs=6))
    stpool = ctx.enter_context(tc.tile_pool(name="stpool", bufs=2))
    smpool = ctx.enter_context(tc.tile_pool(name="smpool", bufs=2))
    plp = ctx.enter_context(tc.tile_pool(name="plp", bufs=2, space="PSUM"))
    pap = ctx.enter_context(tc.tile_pool(name="pap", bufs=2, space="PSUM"))
    dpool = ctx.enter_context(tc.tile_pool(name="dpool", bufs=1, space="DRAM"))

    q_sb = [qpool.tile([P, S], BF, tag="qT", name=f"q{t}", bufs=HT)
            for t in range(HT)]
    k_sb = [kpool.tile([P, S], BF, tag="kT", name=f"k{t}", bufs=HT)
            for t in range(HT)]
    v_sb = [vpool.tile([P, HL, 66], BF, tag="v66", name=f"v{c}", bufs=KT)
            for c in range(KT)]
    eb_sb = [ebpool.tile([P, SL], BF, tag="eb", name=f"eb{c}", bufs=KT)
             for c in range(KT)]            # qh0 tiles; qh1 pool comes later
    rscr = dpool.tile([2 * H, SL], BF, tag="rscr", name="rscr", bufs=1)
    audram = dpool.tile([DH, SL], BF, tag="audram", name="audram", bufs=1)

    state = {"slot": 0, "eb1": None, "an": None, "wo": None}

    with tc.tile_pool(name="xpool", bufs=NT) as xpool, \
         tc.tile_pool(name="wqpool", bufs=NT) as wqpool:
      with tc.tile_pool(name="ypool", bufs=NT) as ypool, \
           tc.tile_pool(name="wkpool", bufs=NT) as wkpool, \
           tc.tile_pool(name="wvpool", bufs=NT) as wvpool:
        x_sb = [xpool.tile([P, S], BF, tag="xT", name=f"x{t}", bufs=NT)
                for t in range(NT)]
        y_sb = [ypool.tile([P, S], BF, tag="yT", name=f"y{t}", bufs=NT)
                for t in range(NT)]
        wq_sb = [wqpool.tile([P, DH], BF, tag="wq", name=f"wq{t}", bufs=NT)
                 for t in range(NT)]
        wk_sb = [wkpool.tile([P, DH], BF, tag="wk", name=f"wk{t}", bufs=NT)
                 for t in range(NT)]
        wv_sb = [wvpool.tile([P, DH], BF, tag="wv", name=f"wv{t}", bufs=NT)
                 for t in range(NT)]

        # ---- input loads: consumption-priority order, striped over the
        # sync+gpsimd queues; first x tiles on scalar (drains early).
        _q = [0]
        _queues = (nc.sync, nc.gpsimd)

        def dq(out, in_):
            _queues[_q[0] % 2].dma_start(out=out, in_=in_)
            _q[0] += 1

        # first-exp critical set first: q cols 0:512 needs x[:,0:512]+wq
        # block; logits c0-3 need wk block + y[:,0:512]; eb c0 for the mul
        for t in range(NT):
            dq(wq_sb[t][:, 0:P], wqT[t * P:(t + 1) * P, 0:P])
        for t in range(NT):
            if t < 3:
                nc.scalar.dma_start(out=x_sb[t][:, 0:512],
                                    in_=xT[t * P:(t + 1) * P, 0:512])
            else:
                dq(x_sb[t][:, 0:512], xT[t * P:(t + 1) * P, 0:512])
        for t in range(NT):
            dq(wk_sb[t][:, 0:P], wkT[t * P:(t + 1) * P, 0:P])
        for t in range(NT):
            dq(y_sb[t][:, 0:512], yT[t * P:(t + 1) * P, 0:512])
        for c in range(4):
            dq(eb_sb[c], ebT[c * P:(c + 1) * P, 0:SL])
        for t in range(NT):
            dq(x_sb[t][:, 512:1024], xT[t * P:(t + 1) * P, 512:1024])
        for t in range(NT):
            dq(wv_sb[t], wvT[t * P:(t + 1) * P, :])
        for t in range(NT):
            dq(y_sb[t][:, 512:1024], yT[t * P:(t + 1) * P, 512:1024])
        for c in range(4, 8):
            dq(eb_sb[c], ebT[c * P:(c + 1) * P, 0:SL])
        for t in range(NT):
            dq(y_sb[t][:, 1024:2048], yT[t * P:(t + 1) * P, 1024:2048])
        for c in range(8, KT):
            dq(eb_sb[c], ebT[c * P:(c + 1) * P, 0:SL])
        for t in range(NT):
            dq(x_sb[t][:, SL:S], xT[t * P:(t + 1) * P, SL:S])
        for t in range(NT):
            dq(wq_sb[t][:, P:DH], wqT[t * P:(t + 1) * P, P:DH])
        for t in range(NT):
            dq(wk_sb[t][:, P:DH], wkT[t * P:(t + 1) * P, P:DH])

        # ---- warm-up heartbeats (HAM p-state ramp), chained to wq then x
        jnk0 = plp.tile([P, 1024], F32, tag="pl", name="jnk0", bufs=2)
        for t in range(NT):
            nc.tensor.matmul(jnk0[0:1, 0:128], lhsT=wq_sb[t][0:1, 0:1],
                             rhs=wq_sb[t][0:1, 0:128], start=True, stop=True)
        for t in range(NT):
            nc.tensor.matmul(jnk0[0:1, 0:512], lhsT=x_sb[t][0:1, 0:1],
                             rhs=x_sb[t][0:1, 0:512], start=True, stop=True)
            if t == 3:
                for _ in range(10):
                    nc.tensor.matmul(jnk0[0:1, 0:512],
                                     lhsT=x_sb[3][0:1, 0:1],
                                     rhs=x_sb[3][0:1, 0:512],
                                     start=True, stop=True)

        # ---- emission helpers ----
        def emit_q_group(t, n0, n1):
            ps = plp.tile([P, 1024], F32, tag="pl", name=f"psq{t}_{n0}",
                          bufs=2)
            for u in range(NT):
                nc.tensor.matmul(ps[:, 0:n1 - n0],
                                 lhsT=wq_sb[u][:, t * P:(t + 1) * P],
                                 rhs=x_sb[u][:, n0:n1],
                                 start=(u == 0), stop=(u == NT - 1))
            nc.vector.tensor_copy(q_sb[t][:, n0:n1], ps[:, 0:n1 - n0])

        def emit_k_group(t, n0, n1):
            ps = plp.tile([P, 1024], F32, tag="pl", name=f"psk{t}_{n0}",
                          bufs=2)
            for u in range(NT):
                nc.tensor.matmul(ps[:, 0:n1 - n0],
                                 lhsT=wk_sb[u][:, t * P:(t + 1) * P],
                                 rhs=y_sb[u][:, n0:n1],
                                 start=(u == 0), stop=(u == NT - 1))
            nc.vector.tensor_copy(k_sb[t][:, n0:n1], ps[:, 0:n1 - n0])

        def emit_v_group(c, g):
            # kv chunk c, d_out group g (256 wide = 4 heads)
            vt = v_sb[c]
            if g == 0:
                nc.vector.memset(vt[:, :, 64:65], 1.0)
                nc.vector.memset(vt[:, :, 65:66], 0.0)
            n0, n1 = g * 256, (g + 1) * 256
            ps = plp.tile([P, 1024], F32, tag="pl", name=f"psv{c}_{g}",
                          bufs=2)
            for u in range(NT):
                nc.tensor.matmul(ps[:, 0:256],
                                 lhsT=y_sb[u][:, c * P:(c + 1) * P],
                                 rhs=wv_sb[u][:, n0:n1],
                                 start=(u == 0), stop=(u == NT - 1))
            src = ps[:, 0:256].rearrange("p (h d) -> p h d", d=DEPTH)
            nc.vector.tensor_copy(vt[:, 4 * g:4 * g + 4, 0:DEPTH], src)

        # 256-wide thunk units (~0.9us) stay under the ~1.07us exp slot
        # period so released PE bursts never starve the ACT stream
        def q_thunks(t, n0=0, n1=S):
            return [lambda a=a, b=b: emit_q_group(t, a, b)
                    for a, b in _chunks(n1 - n0, 256)
                    for a, b in [(a + n0, b + n0)]]

        def k_thunks(t, n0=0, n1=S):
            return [lambda a=a, b=b: emit_k_group(t, a, b)
                    for a, b in _chunks(n1 - n0, 256)
                    for a, b in [(a + n0, b + n0)]]

        def v_thunks(c):
            return [lambda g=g: emit_v_group(c, g) for g in range(2)]

        # ---- prologue: only the two tiles the first exp fronts need.
        # The q[512:1024] chunk and the v-lead (late wv DMA) are emitted
        # AFTER the pre-phase fronts via post_pre, so the first logits
        # gate on just x/y/w first-chunks (~16us) instead of wv (~46us).
        emit_q_group(0, 0, 512)
        emit_k_group(0, 0, 512)

        def post_pre0():
            emit_q_group(0, 512, 1024)
            for c in range(2):
                for th in v_thunks(c):
                    th()

        # ---- the 8 main iterations ----
        def emit_iter(qh, hp, extra_thunks=(), pre=0, fast_tail=False,
                      pre_thunks=(), an_direct=False, post_pre=None):
            q0 = qh * SL
            ha, hb = 2 * hp, 2 * hp + 1
            eb_cur = eb_sb if qh == 0 else state["eb1"]
            # previous iteration's sau copies: they gate the pattn pool
            # rotation, so they must be emitted before this allocation
            for th in pre_thunks:
                th()
            pattn = [pap.tile([65, SL], F32, tag="pattn",
                              name=f"pa{qh}_{ha + hf}", bufs=2)
                     for hf in range(2)]
            thunks = list(extra_thunks)
            sched = {}
            nsc = max(1, KT - 3)
            for i, th in enumerate(thunks):
                sched.setdefault(i * nsc // max(1, len(thunks)), []).append(th)

            def slot_front(c, n0, n1):
                w = n1 - n0
                plt = plp.tile([P, 1024], F32, tag="pl",
                               name=f"pl{qh}_{hp}_{c}_{n0}", bufs=2)
                nc.tensor.matmul(plt[:, 0:w],
                                 lhsT=k_sb[hp][0:DEPTH, c * P:(c + 1) * P],
                                 rhs=q_sb[hp][0:DEPTH, q0 + n0:q0 + n1],
                                 start=True, stop=True)
                nc.tensor.matmul(plt[:, w:2 * w],
                                 lhsT=k_sb[hp][DEPTH:2 * DEPTH,
                                               c * P:(c + 1) * P],
                                 rhs=q_sb[hp][DEPTH:2 * DEPTH,
                                              q0 + n0:q0 + n1],
                                 start=True, stop=True)
                ew = epool.tile([P, 1024], BF, tag="ew",
                                name=f"ew{hp}_{c}_{n0}", bufs=6)
                # depth**-0.5 folded into the ACT scale port
                nc.scalar.activation(ew[:, 0:2 * w], plt[:, 0:2 * w],
                                     EXP, scale=DEPTH ** -0.5)
                # in-place eb multiply, both heads in one op (stride-0
                # broadcast); a fraction of slots go to the idle Pool
                e3 = ew[:, 0:2 * w].rearrange("p (h w) -> p h w", w=w)
                ebb = (eb_cur[c][:, n0:n1].unsqueeze(1)
                       .broadcast_to([P, 2, w]))
                eng = (nc.gpsimd if (state["slot"] % 16) in POOL_SLOTS
                       else nc.vector)
                eng.tensor_mul(e3, e3, ebb)
                state["slot"] += 1
                return ew

            def emit_attnv(c, n0, n1, ew):
                w = n1 - n0
                nc.tensor.matmul(pattn[0][:, n0:n1],
                                 lhsT=v_sb[c][:, ha, 0:65],
                                 rhs=ew[:, 0:w],
                                 start=(c == 0), stop=(c == KT - 1))
                nc.tensor.matmul(pattn[1][:, n0:n1],
                                 lhsT=v_sb[c][:, hb, 0:65],
                                 rhs=ew[:, w:2 * w],
                                 start=(c == 0), stop=(c == KT - 1))

            # pre-phase: run exp fronts ahead so ACT starts before v ready.
            # n0=0 only - the first fronts then gate on just q[:, 0:512],
            # which arrives ~15us before the full q tile.
            pre_ew = {}
            for c in range(pre):
                pre_ew[(c, 0)] = slot_front(c, 0, 512)
            if post_pre is not None:
                post_pre()

            slotq = []
            pend = []
            for c in range(KT):
                pend.extend(sched.get(c, ()))
                left = max(1, 2 * (KT - 2 - c))
                budget = (max(2, (len(pend) + left - 1) // left)
                          if c < KT - 2 else len(pend))
                for n0, n1 in _chunks(SL, 512):
                    ew = pre_ew.pop((c, n0), None)
                    if ew is None:
                        ew = slot_front(c, n0, n1)
                    slotq.append((c, n0, n1, ew))
                    if len(slotq) > 3:
                        emit_attnv(*slotq.pop(0))
                    for th in pend[:budget]:
                        th()
                    del pend[:budget]
            for th in pend:
                th()
            for args in slotq:
                emit_attnv(*args)

            # ---- normalize: sau bf16 copy, denominator row to partitions
            # 0:2 via DMA, one reciprocal for both heads, partition
            # broadcast, 2x multiply.  qh0 rows bounce via audram.
            # Emitted as THUNKS released early in the NEXT iteration's
            # c-loop, so this ~6.5us DVE burst does not sit in front of
            # the next iteration's eb-muls in the DVE queue (sau copies
            # first - they gate the pattn pool rotation).
            den_t = smpool.tile([2, SL], BF, tag="dent", name=f"dn{qh}{hp}",
                                bufs=1)
            saus = [stpool.tile([65, SL], BF, tag="sau",
                                name=f"sa{qh}_{ha + hf}", bufs=2)
                    for hf in range(2)]
            row = qh * H + 2 * hp

            def sau_copy(hf):
                nc.vector.tensor_copy(saus[hf], pattn[hf])
                nc.sync.dma_start(out=den_t[hf:hf + 1, :],
                                  in_=saus[hf][64:65, :])

            def recip_chain():
                denf = smpool.tile([2, SL], F32, tag="denf",
                                   name=f"df{qh}{hp}", bufs=1)
                nc.vector.tensor_copy(denf, den_t)
                recipf = smpool.tile([2, SL], F32, tag="recipf",
                                     name=f"rf{qh}{hp}", bufs=1)
                nc.vector.reciprocal_approx_fast(recipf, denf)
                recipb = smpool.tile([2, SL], BF, tag="recipb",
                                     name=f"rb{qh}{hp}", bufs=1)
                nc.vector.tensor_copy(recipb, recipf)
                nc.sync.dma_start(out=rscr[row:row + 2, :], in_=recipb)

            def bc_mul(hf):
                bc = smpool.tile([DEPTH, SL], BF, tag="bc",
                                 name=f"bc{qh}_{ha + hf}", bufs=2)
                nc.sync.dma_start(
                    out=bc,
                    in_=rscr[row + hf:row + hf + 1, :].partition_broadcast(
                        DEPTH))
                if qh == 0 and not an_direct:
                    anh = smpool.tile([DEPTH, SL], BF, tag="anh",
                                      name=f"ah{ha + hf}", bufs=2)
                    nc.vector.tensor_mul(anh, saus[hf][0:64, :], bc)
                    nc.sync.dma_start(
                        out=audram[hp * P + hf * DEPTH:
                                   hp * P + (hf + 1) * DEPTH, :],
                        in_=anh)
                else:
                    an_sb = state["an"]
                    nc.vector.tensor_mul(
                        an_sb[hp][hf * DEPTH:(hf + 1) * DEPTH, q0:q0 + SL],
                        saus[hf][0:64, :], bc)

            if not fast_tail:
                return [lambda: sau_copy(0), lambda: sau_copy(1),
                        recip_chain, lambda: bc_mul(0), lambda: bc_mul(1)]

            # last iteration: fully on-chip normalize - PE gathers the
            # denominator row and broadcasts the reciprocal, no DRAM
            # round trips on the critical tail.
            ones = smpool.tile([P, DEPTH], BF, tag="ones", name="ones7",
                               bufs=1)
            nc.vector.memset(ones, 1.0)
            for hf in range(2):
                nc.vector.tensor_copy(saus[hf][64:65, :],
                                      pattn[hf][64:65, :])
                nc.vector.tensor_copy(saus[hf][0:64, :], pattn[hf][0:64, :])
            recips = []
            for hf in range(2):
                dps = plp.tile([P, 1024], F32, tag="pl", name=f"dps{hf}",
                               bufs=2)
                for n0, n1 in _chunks(SL, 512):
                    nc.tensor.matmul(dps[0:1, n0:n1],
                                     lhsT=ones[64:65, 0:1],
                                     rhs=saus[hf][64:65, n0:n1],
                                     start=True, stop=True)
                rcf = smpool.tile([2, SL], F32,
                                  tag="recipf" if hf == 0 else "denf",
                                  name=f"rcf7_{hf}", bufs=1)
                nc.vector.reciprocal_approx_fast(rcf[0:1, :], dps[0:1, :])
                rcb = smpool.tile([2, SL], BF,
                                  tag="recipb" if hf == 0 else "dent",
                                  name=f"rcb7_{hf}", bufs=1)
                nc.vector.tensor_copy(rcb[0:1, :], rcf[0:1, :])
                recips.append(rcb)
            an_sb = state["an"]
            for hf in range(2):
                bcp = plp.tile([P, 1024], F32, tag="pl", name=f"bcp{hf}",
                               bufs=2)
                for n0, n1 in _chunks(SL, 512):
                    nc.tensor.matmul(bcp[0:DEPTH, n0:n1],
                                     lhsT=ones[0:1, 0:DEPTH],
                                     rhs=recips[hf][0:1, n0:n1],
                                     start=True, stop=True)
                nc.vector.tensor_mul(
                    an_sb[hp][hf * DEPTH:(hf + 1) * DEPTH, q0:q0 + SL],
                    saus[hf][0:64, :], bcp[0:DEPTH, :])
            return []

        # iterations 0..2 run inside the full pool scope.  q projections
        # emit only the qh0 columns here; the qh1 halves run in it3 when
        # the PE has slack and ACT pacing is established.
        it0_thunks = k_thunks(0, 512, S) + v_thunks(2) + v_thunks(3)
        for c in range(4, KT):
            it0_thunks += v_thunks(c)
        it0_thunks += q_thunks(1, 0, SL) + k_thunks(1)
        nrm = emit_iter(0, 0, extra_thunks=it0_thunks, pre=6,
                        post_pre=post_pre0)
        nrm = emit_iter(0, 1, pre_thunks=nrm[0:2], pre=2,
                        extra_thunks=(nrm[2:] + q_thunks(2, 0, SL)
                                      + k_thunks(2) + q_thunks(3, 0, SL)))
        nrm = emit_iter(0, 2, pre_thunks=nrm[0:2], pre=2,
                        extra_thunks=nrm[2:] + k_thunks(3))

      # y/wk/wv closed; it3 runs the deferred qh1-half q projections
      # (x/wq still resident).
      q1thunks = (q_thunks(0, SL, S) + q_thunks(1, SL, S)
                  + q_thunks(2, SL, S) + q_thunks(3, SL, S))
      nrm = emit_iter(0, 3, pre_thunks=nrm[0:2], pre=2,
                      extra_thunks=nrm[2:] + q1thunks)
      state["nrm"] = nrm

    # x/wq closed: SBUF reused for eb(qh1), Wo, an, o staging.
    eb1pool = ctx.enter_context(tc.tile_pool(name="eb1pool", bufs=KT))
    wopool = ctx.enter_context(tc.tile_pool(name="wopool", bufs=WT))
    anpool = ctx.enter_context(tc.tile_pool(name="anpool", bufs=HT))
    opool = ctx.enter_context(tc.tile_pool(name="opool", bufs=5))
    eb1_sb = [eb1pool.tile([P, SL], BF, tag="eb1", name=f"eb1_{c}", bufs=KT)
              for c in range(KT)]
    wo_sb = [wopool.tile([P, D], BF, tag="wo", name=f"wo{t}", bufs=WT)
             for t in range(WT)]
    an_sb = [anpool.tile([P, S], BF, tag="an", name=f"an{t}", bufs=HT)
             for t in range(HT)]
    for c in range(KT):
        dq(eb1_sb[c], ebT[c * P:(c + 1) * P, SL:S])
    for t in range(WT):
        dq(wo_sb[t], woT[t * P:(t + 1) * P, :])
    # hp0-2's qh0 rows reload now; hp3's audram rows are written by it3's
    # normalize thunks (released inside it4), so its reload is an it4
    # thunk ordered after them.
    for t in range(HT - 1):
        nc.gpsimd.dma_start(out=an_sb[t][:, 0:SL],
                            in_=audram[t * P:(t + 1) * P, :])

    def reload_an3():
        nc.gpsimd.dma_start(out=an_sb[HT - 1][:, 0:SL],
                            in_=audram[(HT - 1) * P:HT * P, :])

    state["eb1"] = eb1_sb
    state["an"] = an_sb

    def emit_o_half(m, q0, cast_eng=None):
        # output rows m*128, query cols q0..q0+512 (one ~1.7us PE burst)
        ps = plp.tile([P, 1024], F32, tag="pl", name=f"pso{m}_{q0}", bufs=2)
        for t in range(WT):
            nc.tensor.matmul(ps[:, 0:512],
                             lhsT=wo_sb[t][:, m * P:(m + 1) * P],
                             rhs=an_sb[t][:, q0:q0 + 512],
                             start=(t == 0), stop=(t == WT - 1))
        osb = opool.tile([P, 512], F32, tag="osb", name=f"o{m}_{q0}",
                         bufs=5)
        if cast_eng is nc.scalar:
            nc.scalar.copy(osb, ps[:, 0:512])
        else:
            nc.vector.tensor_copy(osb, ps[:, 0:512])
        eng = (nc.sync, nc.gpsimd, nc.scalar)[(m + q0 // 512) % 3]
        eng.dma_start(out=outT[m * P:(m + 1) * P, q0:q0 + 512], in_=osb)

    nrm = state["nrm"]
    o0 = [lambda m=m, h=h: emit_o_half(m, h) for m in range(NT)
          for h in (0, 512)]
    # qh0 o-halves cast on ACT: it has structural gaps mid-stream while
    # the DVE queue is the muls' critical path
    o0s = [lambda m=m, h=h: emit_o_half(m, h, cast_eng=nc.scalar)
           for m in range(NT) for h in (0, 512)]
    nrm = emit_iter(1, 0, pre_thunks=nrm[0:2], pre=2,
                    extra_thunks=nrm[2:] + [reload_an3] + o0s[0:2])
    nrm = emit_iter(1, 1, pre_thunks=nrm[0:2], pre=2,
                    extra_thunks=nrm[2:] + o0s[2:9])
    nrm = emit_iter(1, 2, pre_thunks=nrm[0:2], pre=2,
                    extra_thunks=nrm[2:] + o0s[9:16])
    emit_iter(1, 3, pre_thunks=nrm[0:2], pre=2, extra_thunks=nrm[2:],
              fast_tail=True)
    for i, (m, h) in enumerate([(m, h) for m in range(NT)
                                for h in (SL, SL + 512)]):
        emit_o_half(m, h, cast_eng=nc.scalar if i % 2 else nc.vector)

    if DEBUG:
        for t in range(HT):
            nc.sync.dma_start(out=io["anD"][t * P:(t + 1) * P, :],
                              in_=an_sb[t])
            nc.sync.dma_start(out=io["qD"][t * P:(t + 1) * P, :],
                              in_=q_sb[t])
            nc.sync.dma_start(out=io["kD"][t * P:(t + 1) * P, :],
                              in_=k_sb[t])
        nc.sync.dma_start(out=io["rD"], in_=rscr)


def build_nc():
    nc = bacc.Bacc("TRN2", target_bir_lowering=False, debug=False)
    io = {
        "xT": nc.dram_tensor("xT", [D, S], BF, kind="ExternalInput").ap(),
        "yT": nc.dram_tensor("yT", [D, S], BF, kind="ExternalInput").ap(),
        "ebT": nc.dram_tensor("ebT", [S, S], BF, kind="ExternalInput").ap(),
        "wqT": nc.dram_tensor("wqT", [D, DH], BF, kind="ExternalInput").ap(),
        "wkT": nc.dram_tensor("wkT", [D, DH], BF, kind="ExternalInput").ap(),
        "wvT": nc.dram_tensor("wvT", [D, DH], BF, kind="ExternalInput").ap(),
        "woT": nc.dram_tensor("woT", [DH, D], BF, kind="ExternalInput").ap(),
        "outT": nc.dram_tensor("outT", [D, S], F32,
                               kind="ExternalOutput").ap(),
    }
    if DEBUG:
        io["anD"] = nc.dram_tensor("anD", [DH, S], BF,
                                   kind="ExternalOutput").ap()
        io["qD"] = nc.dram_tensor("qD", [DH, S], BF,
                                  kind="ExternalOutput").ap()
        io["kD"] = nc.dram_tensor("kD", [DH, S], BF,
                                  kind="ExternalOutput").ap()
        io["rD"] = nc.dram_tensor("rD", [2 * H, SL], BF,
                                  kind="ExternalOutput").ap()
    with tile.TileContext(nc) as tc:
        with ExitStack() as ctx:
            _attn_body(ctx, tc, io)
    nc.compile()
    return nc


_NC_CACHE = None


def kernel(x, y, bias, Wq, Wk, Wv, Wo):
    global _NC_CACHE, last_exec_time_ns, last_results
    x = np.asarray(x, np.float32)
    y = np.asarray(y, np.float32)
    bias = np.asarray(bias, np.float32)
    Wq, Wk, Wv, Wo = (np.asarray(w, np.float32) for w in (Wq, Wk, Wv, Wo))
    if _NC_CACHE is None:
        _NC_CACHE = build_nc()
    nc = _NC_CACHE

    bf = ml_dtypes.bfloat16
    ebT = np.ascontiguousarray(
        np.exp(bias[0, 0].astype(np.float32)).T).astype(bf)
    xT_all = [np.ascontiguousarray(x[b].T).astype(bf) for b in range(B)]
    yT_all = [np.ascontiguousarray(y[b].T).astype(bf) for b in range(B)]
    wqT = [np.ascontiguousarray(Wq[h * DH:(h + 1) * DH, :].T).astype(bf)
           for h in range(2)]
    wkT = [np.ascontiguousarray(Wk[h * DH:(h + 1) * DH, :].T).astype(bf)
           for h in range(2)]
    wvT = [np.ascontiguousarray(Wv[h * DH:(h + 1) * DH, :].T).astype(bf)
           for h in range(2)]
    woT = [np.ascontiguousarray(Wo.T[h * DH:(h + 1) * DH, :]).astype(bf)
           for h in range(2)]

    in_maps = []
    for core in range(NCORES):
        b, hh = divmod(core, 2)
        in_maps.append({
            "xT": xT_all[b], "yT": yT_all[b], "ebT": ebT,
            "wqT": wqT[hh], "wkT": wkT[hh], "wvT": wvT[hh],
            "woT": woT[hh],
        })

    res = run_bass_kernel_spmd(nc, in_maps, core_ids=list(range(NCORES)),
                               trace=TRACE)
    last_exec_time_ns = res.exec_time_ns
    last_results = res
    out = np.empty((B, S, D), np.float32)
    for b in range(B):
        tot = res.results[2 * b]["outT"] + res.results[2 * b + 1]["outT"]
        out[b] = tot.T
    return out


# revision 63
# speedup vs baseline: 1.1980x; 1.0117x over previous
"""Multi-head attention with bias, distributed over 8 trn2 NeuronCores.

Reference computation (per batch b):
    q = (x @ Wq.T) * depth**-0.5 ; k = y @ Wk.T ; v = y @ Wv.T     (per-head split)
    out = softmax(q @ k.T + bias) @ v @ Wo.T

Sharding: 8 cores = 4 batches x 2 head-halves (tensor parallel over heads).
Core c handles batch b = c//2 and heads (c%2)*8 .. +8, all 2048 queries.
Wq/Wk/Wv column-split, Wo row-split; each core emits a PARTIAL output
[D, S] f32 and the host sums the two halves per batch (the "all-reduce").
vs. a query-split this removes the redundant k/v projections (~60us of PE
per core).

Everything bf16 on the PE (fp8 was tried for the attnv with DoubleRow -
2x faster on the PE - but weight quantization error does NOT average down
through softmax: attn and its error shrink by the same sqrt(n_eff), so
fp8 costs ~4-5% rel err, over the 2e-2 budget).

Device-side layout (feature dim on partitions):
    qT/kT [512, 2048] bf16; logits via K=64 quadrant matmuls (2 heads of a
    128-partition tile run concurrently) into f32 PSUM [128 kk, 2x512 q].
    exp: ACT reads PSUM, scale=1/8 folded in, bf16 out; the exp(bias)
    multiply is one in-place DVE/Pool op per slot (stride-0 head
    broadcast, 2-byte operands keep the DVE 2x fast path; a fraction of
    slots go to the otherwise-idle Pool engine).
    attnT_h(+denom row 64) = [v_h | ones].T @ expw   (K=128, M=65)
    normalize: pattn -> sau bf16, denom row out via DMA, one reciprocal
    for both heads, DMA partition-broadcast back, 2x DVE multiply.
    qh0 normalized rows bounce through DRAM (audram) into an[] during
    qh1 (SBUF is fully booked before the x/y/weight pools close).
    out: Wo.T-projection of an chunks, f32, DMAd as produced; the qh0
    half is thunked into qh1's main loop (PE slack), only qh1 drains at
    the tail.

Main loop: 8 iterations = 2 query-halves x 4 head-pairs.  ScalarE exp
(33.5M elems/core at ~124 G elem/s = 270us) is the pacing engine; PE
slack carries the projection thunks; input loads striped over the
sync+gpsimd DMA queues in consumption order, a few early tiles on
scalar.
"""

import numpy as np
import ml_dtypes
from contextlib import ExitStack

import concourse.bass as bass
import concourse.mybir as mybir
import concourse.tile as tile
from concourse import bacc
from concourse.bass_utils import run_bass_kernel_spmd

# full-problem dims (hardcoded per spec)
B, S, D, H = 4, 2048, 1024, 16
DEPTH = D // H            # 64
P = 128
NCORES = 8

HL = H // 2               # 8 local heads per core
DH = HL * DEPTH           # 512 local head dim
NT = D // P               # 8 d_in tiles
WT = DH // P              # 4 weight-out tiles
HT = WT                   # 4 head-pair tiles
KT = S // P               # 16 kv chunks
SL = S // 2               # 1024 q cols per iteration

BF = mybir.dt.bfloat16
F32 = mybir.dt.float32
EXP = mybir.ActivationFunctionType.Exp

TRACE = False
DEBUG = False
last_exec_time_ns = None
last_results = None

# Pool/DVE share SBUF ports: offloading eb-muls to Pool slowed BOTH
# (DVE 650->918ns, Pool 2169ns) and each 2.2us Pool mul stalled the
# in-order PE stream at its attnv.  All eb-muls stay on DVE.
POOL_SLOTS = frozenset()


def _chunks(total, step):
    return [(n0, min(n0 + step, total)) for n0 in range(0, total, step)]


def _attn_body(ctx, tc, io):
    nc = tc.nc
    xT, yT, ebT, wqT, wkT, wvT, woT, outT = (
        io[k] for k in ("xT", "yT", "ebT", "wqT", "wkT", "wvT", "woT",
                        "outT"))

    # ---- persistent pools ----
    ebpool = ctx.enter_context(tc.tile_pool(name="ebpool", bufs=KT))
    qpool = ctx.enter_context(tc.tile_pool(name="qpool", bufs=HT))
    kpool = ctx.enter_context(tc.tile_pool(name="kpool", bufs=HT))
    vpool = ctx.enter_context(tc.tile_pool(name="vpool", bufs=KT))
    epool = ctx.enter_context(tc.tile_pool(name="epool", bufs=7))
    stpool = ctx.enter_context(tc.tile_pool(name="stpool", bufs=2))
    smpool = ctx.enter_context(tc.tile_pool(name="smpool", bufs=2))
    plp = ctx.enter_context(tc.tile_pool(name="plp", bufs=2, space="PSUM"))
    pap = ctx.enter_context(tc.tile_pool(name="pap", bufs=2, space="PSUM"))
    dpool = ctx.enter_context(tc.tile_pool(name="dpool", bufs=1, space="DRAM"))

    q_sb = [qpool.tile([P, S], BF, tag="qT", name=f"q{t}", bufs=HT)
            for t in range(HT)]
    k_sb = [kpool.tile([P, S], BF, tag="kT", name=f"k{t}", bufs=HT)
            for t in range(HT)]
    v_sb = [vpool.tile([P, HL, 66], BF, tag="v66", name=f"v{c}", bufs=KT)
            for c in range(KT)]
    eb_sb = [ebpool.tile([P, SL], BF, tag="eb", name=f"eb{c}", bufs=KT)
             for c in range(KT)]            # qh0 tiles; qh1 pool comes later
    rscr = dpool.tile([2 * H, SL], BF, tag="rscr", name="rscr", bufs=1)
    audram = dpool.tile([DH, SL], BF, tag="audram", name="audram", bufs=1)

    state = {"slot": 0, "eb1": None, "an": None, "wo": None}

    with tc.tile_pool(name="xpool", bufs=NT) as xpool, \
         tc.tile_pool(name="wqpool", bufs=NT) as wqpool:
      with tc.tile_pool(name="ypool", bufs=NT) as ypool, \
           tc.tile_pool(name="wkpool", bufs=NT) as wkpool, \
           tc.tile_pool(name="wvpool", bufs=NT) as wvpool:
        x_sb = [xpool.tile([P, S], BF, tag="xT", name=f"x{t}", bufs=NT)
                for t in range(NT)]
        y_sb = [ypool.tile([P, S], BF, tag="yT", name=f"y{t}", bufs=NT)
                for t in range(NT)]
        wq_sb = [wqpool.tile([P, DH], BF, tag="wq", name=f"wq{t}", bufs=NT)
                 for t in range(NT)]
        wk_sb = [wkpool.tile([P, DH], BF, tag="wk", name=f"wk{t}", bufs=NT)
                 for t in range(NT)]
        wv_sb = [wvpool.tile([P, DH], BF, tag="wv", name=f"wv{t}", bufs=NT)
                 for t in range(NT)]

        # ---- input loads: consumption-priority order, striped over the
        # sync+gpsimd queues; first x tiles on scalar (drains early).
        _q = [0]
        _queues = (nc.sync, nc.gpsimd)

        def dq(out, in_):
            _queues[_q[0] % 2].dma_start(out=out, in_=in_)
            _q[0] += 1

        # first-exp critical set first: q cols 0:512 needs x[:,0:512]+wq
        # block; logits c0-3 need wk block + y[:,0:512]; eb c0 for the mul
        for t in range(NT):
            dq(wq_sb[t][:, 0:P], wqT[t * P:(t + 1) * P, 0:P])
        for t in range(NT):
            if t < 3:
                nc.scalar.dma_start(out=x_sb[t][:, 0:512],
                                    in_=xT[t * P:(t + 1) * P, 0:512])
            else:
                dq(x_sb[t][:, 0:512], xT[t * P:(t + 1) * P, 0:512])
        for t in range(NT):
            dq(wk_sb[t][:, 0:P], wkT[t * P:(t + 1) * P, 0:P])
        for t in range(NT):
            dq(y_sb[t][:, 0:512], yT[t * P:(t + 1) * P, 0:512])
        for c in range(4):
            dq(eb_sb[c], ebT[c * P:(c + 1) * P, 0:SL])
        for t in range(NT):
            dq(x_sb[t][:, 512:1024], xT[t * P:(t + 1) * P, 512:1024])
        for t in range(NT):
            dq(wv_sb[t], wvT[t * P:(t + 1) * P, :])
        for t in range(NT):
            # scalar ring is drained by now; 4 mid-priority y tiles on it
            # relieve the sync/gpsimd critical prefix
            if t < 4:
                nc.scalar.dma_start(out=y_sb[t][:, 512:1024],
                                    in_=yT[t * P:(t + 1) * P, 512:1024])
            else:
                dq(y_sb[t][:, 512:1024], yT[t * P:(t + 1) * P, 512:1024])
        for c in range(4, 8):
            dq(eb_sb[c], ebT[c * P:(c + 1) * P, 0:SL])
        for t in range(NT):
            dq(y_sb[t][:, 1024:2048], yT[t * P:(t + 1) * P, 1024:2048])
        for c in range(8, KT):
            dq(eb_sb[c], ebT[c * P:(c + 1) * P, 0:SL])
        for t in range(NT):
            dq(x_sb[t][:, SL:S], xT[t * P:(t + 1) * P, SL:S])
        for t in range(NT):
            dq(wq_sb[t][:, P:DH], wqT[t * P:(t + 1) * P, P:DH])
        for t in range(NT):
            dq(wk_sb[t][:, P:DH], wkT[t * P:(t + 1) * P, P:DH])

        # ---- warm-up heartbeats (HAM p-state ramp), chained to wq then x
        jnk0 = plp.tile([P, 1024], F32, tag="pl", name="jnk0", bufs=2)
        for t in range(NT):
            nc.tensor.matmul(jnk0[0:1, 0:128], lhsT=wq_sb[t][0:1, 0:1],
                             rhs=wq_sb[t][0:1, 0:128], start=True, stop=True)
        for t in range(NT):
            nc.tensor.matmul(jnk0[0:1, 0:512], lhsT=x_sb[t][0:1, 0:1],
                             rhs=x_sb[t][0:1, 0:512], start=True, stop=True)
            if t == 3:
                for _ in range(10):
                    nc.tensor.matmul(jnk0[0:1, 0:512],
                                     lhsT=x_sb[3][0:1, 0:1],
                                     rhs=x_sb[3][0:1, 0:512],
                                     start=True, stop=True)

        # ---- emission helpers ----
        def emit_q_group(t, n0, n1):
            ps = plp.tile([P, 1024], F32, tag="pl", name=f"psq{t}_{n0}",
                          bufs=2)
            for u in range(NT):
                nc.tensor.matmul(ps[:, 0:n1 - n0],
                                 lhsT=wq_sb[u][:, t * P:(t + 1) * P],
                                 rhs=x_sb[u][:, n0:n1],
                                 start=(u == 0), stop=(u == NT - 1))
            nc.vector.tensor_copy(q_sb[t][:, n0:n1], ps[:, 0:n1 - n0])

        def emit_k_group(t, n0, n1):
            ps = plp.tile([P, 1024], F32, tag="pl", name=f"psk{t}_{n0}",
                          bufs=2)
            for u in range(NT):
                nc.tensor.matmul(ps[:, 0:n1 - n0],
                                 lhsT=wk_sb[u][:, t * P:(t + 1) * P],
                                 rhs=y_sb[u][:, n0:n1],
                                 start=(u == 0), stop=(u == NT - 1))
            nc.vector.tensor_copy(k_sb[t][:, n0:n1], ps[:, 0:n1 - n0])

        def emit_v_group(c, g):
            # kv chunk c, d_out group g (256 wide = 4 heads)
            vt = v_sb[c]
            if g == 0:
                nc.vector.memset(vt[:, :, 64:65], 1.0)
                nc.vector.memset(vt[:, :, 65:66], 0.0)
            n0, n1 = g * 256, (g + 1) * 256
            ps = plp.tile([P, 1024], F32, tag="pl", name=f"psv{c}_{g}",
                          bufs=2)
            for u in range(NT):
                nc.tensor.matmul(ps[:, 0:256],
                                 lhsT=y_sb[u][:, c * P:(c + 1) * P],
                                 rhs=wv_sb[u][:, n0:n1],
                                 start=(u == 0), stop=(u == NT - 1))
            src = ps[:, 0:256].rearrange("p (h d) -> p h d", d=DEPTH)
            nc.vector.tensor_copy(vt[:, 4 * g:4 * g + 4, 0:DEPTH], src)

        # 256-wide thunk units (~0.9us) stay under the ~1.07us exp slot
        # period so released PE bursts never starve the ACT stream
        def q_thunks(t, n0=0, n1=S):
            return [lambda a=a, b=b: emit_q_group(t, a, b)
                    for a, b in _chunks(n1 - n0, 256)
                    for a, b in [(a + n0, b + n0)]]

        def k_thunks(t, n0=0, n1=S):
            return [lambda a=a, b=b: emit_k_group(t, a, b)
                    for a, b in _chunks(n1 - n0, 256)
                    for a, b in [(a + n0, b + n0)]]

        def v_thunks(c):
            return [lambda g=g: emit_v_group(c, g) for g in range(2)]

        # ---- prologue: only the two tiles the first exp fronts need.
        # The q[512:1024] chunk and the v-lead (late wv DMA) are emitted
        # AFTER the pre-phase fronts via post_pre, so the first logits
        # gate on just x/y/w first-chunks (~16us) instead of wv (~46us).
        emit_q_group(0, 0, 512)
        emit_k_group(0, 0, 512)

        def post_pre0():
            emit_q_group(0, 512, 1024)
            for c in range(2):
                for th in v_thunks(c):
                    th()

        # ---- the 8 main iterations ----
        def emit_iter(qh, hp, extra_thunks=(), pre=0, fast_tail=False,
                      pre_thunks=(), an_direct=False, post_pre=None):
            q0 = qh * SL
            ha, hb = 2 * hp, 2 * hp + 1
            eb_cur = eb_sb if qh == 0 else state["eb1"]
            # previous iteration's sau copies: they gate the pattn pool
            # rotation, so they must be emitted before this allocation
            for th in pre_thunks:
                th()
            pattn = [pap.tile([65, SL], F32, tag="pattn",
                              name=f"pa{qh}_{ha + hf}", bufs=2)
                     for hf in range(2)]
            thunks = list(extra_thunks)
            sched = {}
            nsc = max(1, KT - 3)
            for i, th in enumerate(thunks):
                sched.setdefault(i * nsc // max(1, len(thunks)), []).append(th)

            def slot_front(c, n0, n1):
                w = n1 - n0
                plt = plp.tile([P, 1024], F32, tag="pl",
                               name=f"pl{qh}_{hp}_{c}_{n0}", bufs=2)
                nc.tensor.matmul(plt[:, 0:w],
                                 lhsT=k_sb[hp][0:DEPTH, c * P:(c + 1) * P],
                                 rhs=q_sb[hp][0:DEPTH, q0 + n0:q0 + n1],
                                 start=True, stop=True)
                nc.tensor.matmul(plt[:, w:2 * w],
                                 lhsT=k_sb[hp][DEPTH:2 * DEPTH,
                                               c * P:(c + 1) * P],
                                 rhs=q_sb[hp][DEPTH:2 * DEPTH,
                                              q0 + n0:q0 + n1],
                                 start=True, stop=True)
                ew = epool.tile([P, 1024], BF, tag="ew",
                                name=f"ew{hp}_{c}_{n0}", bufs=7)
                # depth**-0.5 folded into the ACT scale port
                nc.scalar.activation(ew[:, 0:2 * w], plt[:, 0:2 * w],
                                     EXP, scale=DEPTH ** -0.5)
                # in-place eb multiply, both heads in one op (stride-0
                # broadcast); a fraction of slots go to the idle Pool
                e3 = ew[:, 0:2 * w].rearrange("p (h w) -> p h w", w=w)
                ebb = (eb_cur[c][:, n0:n1].unsqueeze(1)
                       .broadcast_to([P, 2, w]))
                eng = (nc.gpsimd if (state["slot"] % 16) in POOL_SLOTS
                       else nc.vector)
                eng.tensor_mul(e3, e3, ebb)
                state["slot"] += 1
                return ew

            def emit_attnv(c, n0, n1, ew):
                w = n1 - n0
                nc.tensor.matmul(pattn[0][:, n0:n1],
                                 lhsT=v_sb[c][:, ha, 0:65],
                                 rhs=ew[:, 0:w],
                                 start=(c == 0), stop=(c == KT - 1))
                nc.tensor.matmul(pattn[1][:, n0:n1],
                                 lhsT=v_sb[c][:, hb, 0:65],
                                 rhs=ew[:, w:2 * w],
                                 start=(c == 0), stop=(c == KT - 1))

            def emit_attnv_pair(s0, s1):
                # both chunks of one kv-chunk: per head the two streams
                # run back-to-back under one weight set
                (c, n0, n1, ew0), (_, m0, m1, ew1) = s0, s1
                for hf, h in ((0, ha), (1, hb)):
                    nc.tensor.matmul(pattn[hf][:, n0:n1],
                                     lhsT=v_sb[c][:, h, 0:65],
                                     rhs=ew0[:, 0:512] if hf == 0
                                     else ew0[:, 512:1024],
                                     start=(c == 0), stop=(c == KT - 1))
                    nc.tensor.matmul(pattn[hf][:, m0:m1],
                                     lhsT=v_sb[c][:, h, 0:65],
                                     rhs=ew1[:, 0:512] if hf == 0
                                     else ew1[:, 512:1024],
                                     start=(c == 0), stop=(c == KT - 1))

            # pre-phase: run exp fronts ahead so ACT starts before v ready.
            # n0=0 only - the first fronts then gate on just q[:, 0:512],
            # which arrives ~15us before the full q tile.
            pre_ew = {}
            for c in range(pre):
                pre_ew[(c, 0)] = slot_front(c, 0, 512)
            if post_pre is not None:
                post_pre()

            slotq = []
            pend = []
            for c in range(KT):
                pend.extend(sched.get(c, ()))
                left = max(1, 2 * (KT - 2 - c))
                budget = (max(2, (len(pend) + left - 1) // left)
                          if c < KT - 2 else len(pend))
                for n0, n1 in _chunks(SL, 512):
                    ew = pre_ew.pop((c, n0), None)
                    if ew is None:
                        ew = slot_front(c, n0, n1)
                    slotq.append((c, n0, n1, ew))
                    if n0 == 512 and len(slotq) > 4:
                        s0 = slotq.pop(0)
                        s1 = slotq.pop(0)
                        emit_attnv_pair(s0, s1)
                    for th in pend[:budget]:
                        th()
                    del pend[:budget]
            for th in pend:
                th()
            while len(slotq) >= 2:
                emit_attnv_pair(slotq.pop(0), slotq.pop(0))
            for args in slotq:
                emit_attnv(*args)

            # ---- normalize: sau bf16 copy, denominator row to partitions
            # 0:2 via DMA, one reciprocal for both heads, partition
            # broadcast, 2x multiply.  qh0 rows bounce via audram.
            # Emitted as THUNKS released early in the NEXT iteration's
            # c-loop, so this ~6.5us DVE burst does not sit in front of
            # the next iteration's eb-muls in the DVE queue (sau copies
            # first - they gate the pattn pool rotation).
            den_t = smpool.tile([2, SL], BF, tag="dent", name=f"dn{qh}{hp}",
                                bufs=1)
            saus = [stpool.tile([65, SL], BF, tag="sau",
                                name=f"sa{qh}_{ha + hf}", bufs=2)
                    for hf in range(2)]
            row = qh * H + 2 * hp

            def sau_copy(hf):
                nc.vector.tensor_copy(saus[hf], pattn[hf])
                nc.sync.dma_start(out=den_t[hf:hf + 1, :],
                                  in_=saus[hf][64:65, :])

            def recip_chain():
                denf = smpool.tile([2, SL], F32, tag="denf",
                                   name=f"df{qh}{hp}", bufs=1)
                nc.vector.tensor_copy(denf, den_t)
                recipf = smpool.tile([2, SL], F32, tag="recipf",
                                     name=f"rf{qh}{hp}", bufs=1)
                nc.vector.reciprocal_approx_fast(recipf, denf)
                recipb = smpool.tile([2, SL], BF, tag="recipb",
                                     name=f"rb{qh}{hp}", bufs=1)
                nc.vector.tensor_copy(recipb, recipf)
                nc.sync.dma_start(out=rscr[row:row + 2, :], in_=recipb)

            def bc_mul(hf):
                bc = smpool.tile([DEPTH, SL], BF, tag="bc",
                                 name=f"bc{qh}_{ha + hf}", bufs=2)
                nc.sync.dma_start(
                    out=bc,
                    in_=rscr[row + hf:row + hf + 1, :].partition_broadcast(
                        DEPTH))
                if qh == 0 and not an_direct:
                    anh = smpool.tile([DEPTH, SL], BF, tag="anh",
                                      name=f"ah{ha + hf}", bufs=2)
                    nc.vector.tensor_mul(anh, saus[hf][0:64, :], bc)
                    nc.sync.dma_start(
                        out=audram[hp * P + hf * DEPTH:
                                   hp * P + (hf + 1) * DEPTH, :],
                        in_=anh)
                else:
                    an_sb = state["an"]
                    nc.vector.tensor_mul(
                        an_sb[hp][hf * DEPTH:(hf + 1) * DEPTH, q0:q0 + SL],
                        saus[hf][0:64, :], bc)

            if not fast_tail:
                return [lambda: sau_copy(0), lambda: sau_copy(1),
                        recip_chain, lambda: bc_mul(0), lambda: bc_mul(1)]

            # last iteration: fully on-chip normalize - PE gathers the
            # denominator row and broadcasts the reciprocal, no DRAM
            # round trips on the critical tail.
            ones = smpool.tile([P, DEPTH], BF, tag="ones", name="ones7",
                               bufs=1)
            nc.vector.memset(ones, 1.0)
            for hf in range(2):
                nc.vector.tensor_copy(saus[hf][64:65, :],
                                      pattn[hf][64:65, :])
                nc.vector.tensor_copy(saus[hf][0:64, :], pattn[hf][0:64, :])
            recips = []
            for hf in range(2):
                dps = plp.tile([P, 1024], F32, tag="pl", name=f"dps{hf}",
                               bufs=2)
                for n0, n1 in _chunks(SL, 512):
                    nc.tensor.matmul(dps[0:1, n0:n1],
                                     lhsT=ones[64:65, 0:1],
                                     rhs=saus[hf][64:65, n0:n1],
                                     start=True, stop=True)
                rcf = smpool.tile([2, SL], F32,
                                  tag="recipf" if hf == 0 else "denf",
                                  name=f"rcf7_{hf}", bufs=1)
                nc.vector.reciprocal_approx_fast(rcf[0:1, :], dps[0:1, :])
                rcb = smpool.tile([2, SL], BF,
                                  tag="recipb" if hf == 0 else "dent",
                                  name=f"rcb7_{hf}", bufs=1)
                nc.vector.tensor_copy(rcb[0:1, :], rcf[0:1, :])
                recips.append(rcb)
            an_sb = state["an"]
            for hf in range(2):
                bcp = plp.tile([P, 1024], F32, tag="pl", name=f"bcp{hf}",
                               bufs=2)
                for n0, n1 in _chunks(SL, 512):
                    nc.tensor.matmul(bcp[0:DEPTH, n0:n1],
                                     lhsT=ones[0:1, 0:DEPTH],
                                     rhs=recips[hf][0:1, n0:n1],
                                     start=True, stop=True)
                nc.vector.tensor_mul(
                    an_sb[hp][hf * DEPTH:(hf + 1) * DEPTH, q0:q0 + SL],
                    saus[hf][0:64, :], bcp[0:DEPTH, :])
            return []

        # iterations 0..2 run inside the full pool scope.  q projections
        # emit only the qh0 columns here; the qh1 halves run in it3 when
        # the PE has slack and ACT pacing is established.
        it0_thunks = k_thunks(0, 512, S) + v_thunks(2) + v_thunks(3)
        for c in range(4, KT):
            it0_thunks += v_thunks(c)
        it0_thunks += q_thunks(1, 0, SL) + k_thunks(1)
        nrm = emit_iter(0, 0, extra_thunks=it0_thunks, pre=6,
                        post_pre=post_pre0)
        nrm = emit_iter(0, 1, pre_thunks=nrm[0:2], pre=3,
                        extra_thunks=(nrm[2:] + q_thunks(2, 0, SL)
                                      + k_thunks(2) + q_thunks(3, 0, SL)))
        nrm = emit_iter(0, 2, pre_thunks=nrm[0:2], pre=3,
                        extra_thunks=nrm[2:] + k_thunks(3))

      # y/wk/wv closed; it3 runs the deferred qh1-half q projections
      # (x/wq still resident).
      q1thunks = (q_thunks(0, SL, S) + q_thunks(1, SL, S)
                  + q_thunks(2, SL, S) + q_thunks(3, SL, S))
      nrm = emit_iter(0, 3, pre_thunks=nrm[0:2], pre=3,
                      extra_thunks=nrm[2:] + q1thunks)
      state["nrm"] = nrm

    # x/wq closed: SBUF reused for eb(qh1), Wo, an, o staging.
    eb1pool = ctx.enter_context(tc.tile_pool(name="eb1pool", bufs=KT))
    wopool = ctx.enter_context(tc.tile_pool(name="wopool", bufs=WT))
    anpool = ctx.enter_context(tc.tile_pool(name="anpool", bufs=HT))
    opool = ctx.enter_context(tc.tile_pool(name="opool", bufs=5))
    eb1_sb = [eb1pool.tile([P, SL], BF, tag="eb1", name=f"eb1_{c}", bufs=KT)
              for c in range(KT)]
    wo_sb = [wopool.tile([P, D], BF, tag="wo", name=f"wo{t}", bufs=WT)
             for t in range(WT)]
    an_sb = [anpool.tile([P, S], BF, tag="an", name=f"an{t}", bufs=HT)
             for t in range(HT)]
    for c in range(KT):
        dq(eb1_sb[c], ebT[c * P:(c + 1) * P, SL:S])
    for t in range(WT):
        dq(wo_sb[t], woT[t * P:(t + 1) * P, :])
    # hp0-2's qh0 rows reload now; hp3's audram rows are written by it3's
    # normalize thunks (released inside it4), so its reload is an it4
    # thunk ordered after them.
    for t in range(HT - 1):
        nc.gpsimd.dma_start(out=an_sb[t][:, 0:SL],
                            in_=audram[t * P:(t + 1) * P, :])

    def reload_an3():
        nc.gpsimd.dma_start(out=an_sb[HT - 1][:, 0:SL],
                            in_=audram[(HT - 1) * P:HT * P, :])

    state["eb1"] = eb1_sb
    state["an"] = an_sb

    def emit_o_half(m, q0, cast_eng=None):
        # output rows m*128, query cols q0..q0+512 (one ~1.7us PE burst)
        ps = plp.tile([P, 1024], F32, tag="pl", name=f"pso{m}_{q0}", bufs=2)
        for t in range(WT):
            nc.tensor.matmul(ps[:, 0:512],
                             lhsT=wo_sb[t][:, m * P:(m + 1) * P],
                             rhs=an_sb[t][:, q0:q0 + 512],
                             start=(t == 0), stop=(t == WT - 1))
        osb = opool.tile([P, 512], F32, tag="osb", name=f"o{m}_{q0}",
                         bufs=5)
        if cast_eng is nc.scalar:
            nc.scalar.copy(osb, ps[:, 0:512])
        else:
            nc.vector.tensor_copy(osb, ps[:, 0:512])
        eng = (nc.sync, nc.gpsimd, nc.scalar)[(m + q0 // 512) % 3]
        eng.dma_start(out=outT[m * P:(m + 1) * P, q0:q0 + 512], in_=osb)

    nrm = state["nrm"]
    o0 = [lambda m=m, h=h: emit_o_half(m, h) for m in range(NT)
          for h in (0, 512)]
    # qh0 o-halves cast on ACT: it has structural gaps mid-stream while
    # the DVE queue is the muls' critical path
    o0s = [lambda m=m, h=h: emit_o_half(m, h, cast_eng=nc.scalar)
           for m in range(NT) for h in (0, 512)]
    nrm = emit_iter(1, 0, pre_thunks=nrm[0:2], pre=3,
                    extra_thunks=nrm[2:] + [reload_an3] + o0s[0:2])
    nrm = emit_iter(1, 1, pre_thunks=nrm[0:2], pre=3,
                    extra_thunks=nrm[2:] + o0s[2:9])
    nrm = emit_iter(1, 2, pre_thunks=nrm[0:2], pre=3,
                    extra_thunks=nrm[2:] + o0s[9:16])
    emit_iter(1, 3, pre_thunks=nrm[0:2], pre=3, extra_thunks=nrm[2:],
              fast_tail=True)
    for i, (m, h) in enumerate([(m, h) for m in range(NT)
                                for h in (SL, SL + 512)]):
        emit_o_half(m, h, cast_eng=nc.scalar if i % 2 else nc.vector)

    if DEBUG:
        for t in range(HT):
            nc.sync.dma_start(out=io["anD"][t * P:(t + 1) * P, :],
                              in_=an_sb[t])
            nc.sync.dma_start(out=io["qD"][t * P:(t + 1) * P, :],
                              in_=q_sb[t])
            nc.sync.dma_start(out=io["kD"][t * P:(t + 1) * P, :],
                              in_=k_sb[t])
        nc.sync.dma_start(out=io["rD"], in_=rscr)


def build_nc():
    nc = bacc.Bacc("TRN2", target_bir_lowering=False, debug=False)
    io = {
        "xT": nc.dram_tensor("xT", [D, S], BF, kind="ExternalInput").ap(),
        "yT": nc.dram_tensor("yT", [D, S], BF, kind="ExternalInput").ap(),
        "ebT": nc.dram_tensor("ebT", [S, S], BF, kind="ExternalInput").ap(),
        "wqT": nc.dram_tensor("wqT", [D, DH], BF, kind="ExternalInput").ap(),
        "wkT": nc.dram_tensor("wkT", [D, DH], BF, kind="ExternalInput").ap(),
        "wvT": nc.dram_tensor("wvT", [D, DH], BF, kind="ExternalInput").ap(),
        "woT": nc.dram_tensor("woT", [DH, D], BF, kind="ExternalInput").ap(),
        "outT": nc.dram_tensor("outT", [D, S], F32,
                               kind="ExternalOutput").ap(),
    }
    if DEBUG:
        io["anD"] = nc.dram_tensor("anD", [DH, S], BF,
                                   kind="ExternalOutput").ap()
        io["qD"] = nc.dram_tensor("qD", [DH, S], BF,
                                  kind="ExternalOutput").ap()
        io["kD"] = nc.dram_tensor("kD", [DH, S], BF,
                                  kind="ExternalOutput").ap()
        io["rD"] = nc.dram_tensor("rD", [2 * H, SL], BF,
                                  kind="ExternalOutput").ap()
    with tile.TileContext(nc) as tc:
        with ExitStack() as ctx:
            _attn_body(ctx, tc, io)
    nc.compile()
    return nc


_NC_CACHE = None


def kernel(x, y, bias, Wq, Wk, Wv, Wo):
    global _NC_CACHE, last_exec_time_ns, last_results
    x = np.asarray(x, np.float32)
    y = np.asarray(y, np.float32)
    bias = np.asarray(bias, np.float32)
    Wq, Wk, Wv, Wo = (np.asarray(w, np.float32) for w in (Wq, Wk, Wv, Wo))
    if _NC_CACHE is None:
        _NC_CACHE = build_nc()
    nc = _NC_CACHE

    bf = ml_dtypes.bfloat16
    ebT = np.ascontiguousarray(
        np.exp(bias[0, 0].astype(np.float32)).T).astype(bf)
    xT_all = [np.ascontiguousarray(x[b].T).astype(bf) for b in range(B)]
    yT_all = [np.ascontiguousarray(y[b].T).astype(bf) for b in range(B)]
    wqT = [np.ascontiguousarray(Wq[h * DH:(h + 1) * DH, :].T).astype(bf)
           for h in range(2)]
    wkT = [np.ascontiguousarray(Wk[h * DH:(h + 1) * DH, :].T).astype(bf)
           for h in range(2)]
    wvT = [np.ascontiguousarray(Wv[h * DH:(h + 1) * DH, :].T).astype(bf)
           for h in range(2)]
    woT = [np.ascontiguousarray(Wo.T[h * DH:(h + 1) * DH, :]).astype(bf)
           for h in range(2)]

    in_maps = []
    for core in range(NCORES):
        b, hh = divmod(core, 2)
        in_maps.append({
            "xT": xT_all[b], "yT": yT_all[b], "ebT": ebT,
            "wqT": wqT[hh], "wkT": wkT[hh], "wvT": wvT[hh],
            "woT": woT[hh],
        })

    res = run_bass_kernel_spmd(nc, in_maps, core_ids=list(range(NCORES)),
                               trace=TRACE)
    last_exec_time_ns = res.exec_time_ns
    last_results = res
    out = np.empty((B, S, D), np.float32)
    for b in range(B):
        tot = res.results[2 * b]["outT"] + res.results[2 * b + 1]["outT"]
        out[b] = tot.T
    return out


# revision 65
# speedup vs baseline: 1.2075x; 1.0079x over previous
"""Multi-head attention with bias, distributed over 8 trn2 NeuronCores.

Reference computation (per batch b):
    q = (x @ Wq.T) * depth**-0.5 ; k = y @ Wk.T ; v = y @ Wv.T     (per-head split)
    out = softmax(q @ k.T + bias) @ v @ Wo.T

Sharding: 8 cores = 4 batches x 2 head-halves (tensor parallel over heads).
Core c handles batch b = c//2 and heads (c%2)*8 .. +8, all 2048 queries.
Wq/Wk/Wv column-split, Wo row-split; each core emits a PARTIAL output
[D, S] f32 and the host sums the two halves per batch (the "all-reduce").
vs. a query-split this removes the redundant k/v projections (~60us of PE
per core).

Everything bf16 on the PE (fp8 was tried for the attnv with DoubleRow -
2x faster on the PE - but weight quantization error does NOT average down
through softmax: attn and its error shrink by the same sqrt(n_eff), so
fp8 costs ~4-5% rel err, over the 2e-2 budget).

Device-side layout (feature dim on partitions):
    qT/kT [512, 2048] bf16; logits via K=64 quadrant matmuls (2 heads of a
    128-partition tile run concurrently) into f32 PSUM [128 kk, 2x512 q].
    exp: ACT reads PSUM, scale=1/8 folded in, bf16 out; the exp(bias)
    multiply is one in-place DVE/Pool op per slot (stride-0 head
    broadcast, 2-byte operands keep the DVE 2x fast path; a fraction of
    slots go to the otherwise-idle Pool engine).
    attnT_h(+denom row 64) = [v_h | ones].T @ expw   (K=128, M=65)
    normalize: pattn -> sau bf16, denom row out via DMA, one reciprocal
    for both heads, DMA partition-broadcast back, 2x DVE multiply.
    qh0 normalized rows bounce through DRAM (audram) into an[] during
    qh1 (SBUF is fully booked before the x/y/weight pools close).
    out: Wo.T-projection of an chunks, f32, DMAd as produced; the qh0
    half is thunked into qh1's main loop (PE slack), only qh1 drains at
    the tail.

Main loop: 8 iterations = 2 query-halves x 4 head-pairs.  ScalarE exp
(33.5M elems/core at ~124 G elem/s = 270us) is the pacing engine; PE
slack carries the projection thunks; input loads striped over the
sync+gpsimd DMA queues in consumption order, a few early tiles on
scalar.
"""

import numpy as np
import ml_dtypes
from contextlib import ExitStack

import concourse.bass as bass
import concourse.mybir as mybir
import concourse.tile as tile
from concourse import bacc
from concourse.bass_utils import run_bass_kernel_spmd

# full-problem dims (hardcoded per spec)
B, S, D, H = 4, 2048, 1024, 16
DEPTH = D // H            # 64
P = 128
NCORES = 8

HL = H // 2               # 8 local heads per core
DH = HL * DEPTH           # 512 local head dim
NT = D // P               # 8 d_in tiles
WT = DH // P              # 4 weight-out tiles
HT = WT                   # 4 head-pair tiles
KT = S // P               # 16 kv chunks
SL = S // 2               # 1024 q cols per iteration

BF = mybir.dt.bfloat16
F32 = mybir.dt.float32
EXP = mybir.ActivationFunctionType.Exp

TRACE = False
DEBUG = False
last_exec_time_ns = None
last_results = None

# Pool/DVE share SBUF ports: offloading eb-muls to Pool slowed BOTH
# (DVE 650->918ns, Pool 2169ns) and each 2.2us Pool mul stalled the
# in-order PE stream at its attnv.  All eb-muls stay on DVE.
POOL_SLOTS = frozenset()


def _chunks(total, step):
    return [(n0, min(n0 + step, total)) for n0 in range(0, total, step)]


def _attn_body(ctx, tc, io):
    nc = tc.nc
    xT, yT, ebT, wqT, wkT, wvT, woT, outT = (
        io[k] for k in ("xT", "yT", "ebT", "wqT", "wkT", "wvT", "woT",
                        "outT"))

    # ---- persistent pools ----
    ebpool = ctx.enter_context(tc.tile_pool(name="ebpool", bufs=KT))
    qpool = ctx.enter_context(tc.tile_pool(name="qpool", bufs=HT))
    kpool = ctx.enter_context(tc.tile_pool(name="kpool", bufs=HT))
    vpool = ctx.enter_context(tc.tile_pool(name="vpool", bufs=KT))
    epool = ctx.enter_context(tc.tile_pool(name="epool", bufs=7))
    stpool = ctx.enter_context(tc.tile_pool(name="stpool", bufs=2))
    smpool = ctx.enter_context(tc.tile_pool(name="smpool", bufs=2))
    plp = ctx.enter_context(tc.tile_pool(name="plp", bufs=2, space="PSUM"))
    pap = ctx.enter_context(tc.tile_pool(name="pap", bufs=2, space="PSUM"))
    dpool = ctx.enter_context(tc.tile_pool(name="dpool", bufs=1, space="DRAM"))

    q_sb = [qpool.tile([P, S], BF, tag="qT", name=f"q{t}", bufs=HT)
            for t in range(HT)]
    k_sb = [kpool.tile([P, S], BF, tag="kT", name=f"k{t}", bufs=HT)
            for t in range(HT)]
    v_sb = [vpool.tile([P, HL, 66], BF, tag="v66", name=f"v{c}", bufs=KT)
            for c in range(KT)]
    eb_sb = [ebpool.tile([P, SL], BF, tag="eb", name=f"eb{c}", bufs=KT)
             for c in range(KT)]            # qh0 tiles; qh1 pool comes later
    rscr = dpool.tile([2 * H, SL], BF, tag="rscr", name="rscr", bufs=1)
    audram = dpool.tile([DH, SL], BF, tag="audram", name="audram", bufs=1)

    state = {"slot": 0, "eb1": None, "an": None, "wo": None}

    with tc.tile_pool(name="xpool", bufs=NT) as xpool, \
         tc.tile_pool(name="wqpool", bufs=NT) as wqpool:
      with tc.tile_pool(name="ypool", bufs=NT) as ypool, \
           tc.tile_pool(name="wkpool", bufs=NT) as wkpool, \
           tc.tile_pool(name="wvpool", bufs=NT) as wvpool:
        x_sb = [xpool.tile([P, S], BF, tag="xT", name=f"x{t}", bufs=NT)
                for t in range(NT)]
        y_sb = [ypool.tile([P, S], BF, tag="yT", name=f"y{t}", bufs=NT)
                for t in range(NT)]
        wq_sb = [wqpool.tile([P, DH], BF, tag="wq", name=f"wq{t}", bufs=NT)
                 for t in range(NT)]
        wk_sb = [wkpool.tile([P, DH], BF, tag="wk", name=f"wk{t}", bufs=NT)
                 for t in range(NT)]
        wv_sb = [wvpool.tile([P, DH], BF, tag="wv", name=f"wv{t}", bufs=NT)
                 for t in range(NT)]

        # ---- input loads: consumption-priority order, striped over the
        # sync+gpsimd queues; first x tiles on scalar (drains early).
        _q = [0]
        _queues = (nc.sync, nc.gpsimd)

        def dq(out, in_):
            _queues[_q[0] % 2].dma_start(out=out, in_=in_)
            _q[0] += 1

        # first-exp critical set first: q cols 0:512 needs x[:,0:512]+wq
        # block; logits c0-3 need wk block + y[:,0:512]; eb c0 for the mul
        for t in range(NT):
            dq(wq_sb[t][:, 0:P], wqT[t * P:(t + 1) * P, 0:P])
        for t in range(NT):
            if t < 3:
                nc.scalar.dma_start(out=x_sb[t][:, 0:512],
                                    in_=xT[t * P:(t + 1) * P, 0:512])
            else:
                dq(x_sb[t][:, 0:512], xT[t * P:(t + 1) * P, 0:512])
        for t in range(NT):
            dq(wk_sb[t][:, 0:P], wkT[t * P:(t + 1) * P, 0:P])
        for t in range(NT):
            dq(y_sb[t][:, 0:512], yT[t * P:(t + 1) * P, 0:512])
        for c in range(4):
            dq(eb_sb[c], ebT[c * P:(c + 1) * P, 0:SL])
        for t in range(NT):
            dq(x_sb[t][:, 512:1024], xT[t * P:(t + 1) * P, 512:1024])
        for t in range(NT):
            dq(wv_sb[t], wvT[t * P:(t + 1) * P, :])
        for t in range(NT):
            # scalar ring is drained by now; 4 mid-priority y tiles on it
            # relieve the sync/gpsimd critical prefix
            if t < 4:
                nc.scalar.dma_start(out=y_sb[t][:, 512:1024],
                                    in_=yT[t * P:(t + 1) * P, 512:1024])
            else:
                dq(y_sb[t][:, 512:1024], yT[t * P:(t + 1) * P, 512:1024])
        for c in range(4, 8):
            dq(eb_sb[c], ebT[c * P:(c + 1) * P, 0:SL])
        for t in range(NT):
            dq(y_sb[t][:, 1024:2048], yT[t * P:(t + 1) * P, 1024:2048])
        for c in range(8, KT):
            dq(eb_sb[c], ebT[c * P:(c + 1) * P, 0:SL])
        for t in range(NT):
            dq(x_sb[t][:, SL:S], xT[t * P:(t + 1) * P, SL:S])
        for t in range(NT):
            dq(wq_sb[t][:, P:DH], wqT[t * P:(t + 1) * P, P:DH])
        for t in range(NT):
            dq(wk_sb[t][:, P:DH], wkT[t * P:(t + 1) * P, P:DH])

        # ---- warm-up heartbeats (HAM p-state ramp), chained to wq then x
        jnk0 = plp.tile([P, 1024], F32, tag="pl", name="jnk0", bufs=2)
        for t in range(NT):
            nc.tensor.matmul(jnk0[0:1, 0:128], lhsT=wq_sb[t][0:1, 0:1],
                             rhs=wq_sb[t][0:1, 0:128], start=True, stop=True)
        for t in range(NT):
            nc.tensor.matmul(jnk0[0:1, 0:512], lhsT=x_sb[t][0:1, 0:1],
                             rhs=x_sb[t][0:1, 0:512], start=True, stop=True)
            if t == 3:
                for _ in range(10):
                    nc.tensor.matmul(jnk0[0:1, 0:512],
                                     lhsT=x_sb[3][0:1, 0:1],
                                     rhs=x_sb[3][0:1, 0:512],
                                     start=True, stop=True)

        # ---- emission helpers ----
        def emit_q_group(t, n0, n1):
            ps = plp.tile([P, 1024], F32, tag="pl", name=f"psq{t}_{n0}",
                          bufs=2)
            for u in range(NT):
                nc.tensor.matmul(ps[:, 0:n1 - n0],
                                 lhsT=wq_sb[u][:, t * P:(t + 1) * P],
                                 rhs=x_sb[u][:, n0:n1],
                                 start=(u == 0), stop=(u == NT - 1))
            nc.vector.tensor_copy(q_sb[t][:, n0:n1], ps[:, 0:n1 - n0])

        def emit_k_group(t, n0, n1):
            ps = plp.tile([P, 1024], F32, tag="pl", name=f"psk{t}_{n0}",
                          bufs=2)
            for u in range(NT):
                nc.tensor.matmul(ps[:, 0:n1 - n0],
                                 lhsT=wk_sb[u][:, t * P:(t + 1) * P],
                                 rhs=y_sb[u][:, n0:n1],
                                 start=(u == 0), stop=(u == NT - 1))
            nc.vector.tensor_copy(k_sb[t][:, n0:n1], ps[:, 0:n1 - n0])

        def emit_v_group(c, g):
            # kv chunk c, d_out group g (256 wide = 4 heads)
            vt = v_sb[c]
            if g == 0:
                nc.vector.memset(vt[:, :, 64:65], 1.0)
                nc.vector.memset(vt[:, :, 65:66], 0.0)
            n0, n1 = g * 256, (g + 1) * 256
            ps = plp.tile([P, 1024], F32, tag="pl", name=f"psv{c}_{g}",
                          bufs=2)
            for u in range(NT):
                nc.tensor.matmul(ps[:, 0:256],
                                 lhsT=y_sb[u][:, c * P:(c + 1) * P],
                                 rhs=wv_sb[u][:, n0:n1],
                                 start=(u == 0), stop=(u == NT - 1))
            src = ps[:, 0:256].rearrange("p (h d) -> p h d", d=DEPTH)
            nc.vector.tensor_copy(vt[:, 4 * g:4 * g + 4, 0:DEPTH], src)

        # 256-wide thunk units (~0.9us) stay under the ~1.07us exp slot
        # period so released PE bursts never starve the ACT stream
        def q_thunks(t, n0=0, n1=S):
            return [lambda a=a, b=b: emit_q_group(t, a, b)
                    for a, b in _chunks(n1 - n0, 256)
                    for a, b in [(a + n0, b + n0)]]

        def k_thunks(t, n0=0, n1=S):
            return [lambda a=a, b=b: emit_k_group(t, a, b)
                    for a, b in _chunks(n1 - n0, 256)
                    for a, b in [(a + n0, b + n0)]]

        def v_thunks(c):
            return [lambda g=g: emit_v_group(c, g) for g in range(2)]

        # ---- prologue: only the two tiles the first exp fronts need.
        # The q[512:1024] chunk and the v-lead (late wv DMA) are emitted
        # AFTER the pre-phase fronts via post_pre, so the first logits
        # gate on just x/y/w first-chunks (~16us) instead of wv (~46us).
        emit_q_group(0, 0, 512)
        emit_k_group(0, 0, 512)

        def post_pre0():
            emit_q_group(0, 512, 1024)
            for c in range(2):
                for th in v_thunks(c):
                    th()

        # ---- the 8 main iterations ----
        def emit_iter(qh, hp, extra_thunks=(), pre=0, fast_tail=False,
                      pre_thunks=(), an_direct=False, post_pre=None):
            q0 = qh * SL
            ha, hb = 2 * hp, 2 * hp + 1
            eb_cur = eb_sb if qh == 0 else state["eb1"]
            # previous iteration's sau copies: they gate the pattn pool
            # rotation, so they must be emitted before this allocation
            for th in pre_thunks:
                th()
            pattn = [pap.tile([65, SL], F32, tag="pattn",
                              name=f"pa{qh}_{ha + hf}", bufs=2)
                     for hf in range(2)]
            thunks = list(extra_thunks)
            sched = {}
            nsc = max(1, KT - 3)
            for i, th in enumerate(thunks):
                sched.setdefault(i * nsc // max(1, len(thunks)), []).append(th)

            def slot_front(c, n0, n1):
                w = n1 - n0
                plt = plp.tile([P, 1024], F32, tag="pl",
                               name=f"pl{qh}_{hp}_{c}_{n0}", bufs=2)
                nc.tensor.matmul(plt[:, 0:w],
                                 lhsT=k_sb[hp][0:DEPTH, c * P:(c + 1) * P],
                                 rhs=q_sb[hp][0:DEPTH, q0 + n0:q0 + n1],
                                 start=True, stop=True)
                nc.tensor.matmul(plt[:, w:2 * w],
                                 lhsT=k_sb[hp][DEPTH:2 * DEPTH,
                                               c * P:(c + 1) * P],
                                 rhs=q_sb[hp][DEPTH:2 * DEPTH,
                                              q0 + n0:q0 + n1],
                                 start=True, stop=True)
                ew = epool.tile([P, 1024], BF, tag="ew",
                                name=f"ew{hp}_{c}_{n0}", bufs=7)
                # depth**-0.5 folded into the ACT scale port
                nc.scalar.activation(ew[:, 0:2 * w], plt[:, 0:2 * w],
                                     EXP, scale=DEPTH ** -0.5)
                # in-place eb multiply, both heads in one op (stride-0
                # broadcast); a fraction of slots go to the idle Pool
                e3 = ew[:, 0:2 * w].rearrange("p (h w) -> p h w", w=w)
                ebb = (eb_cur[c][:, n0:n1].unsqueeze(1)
                       .broadcast_to([P, 2, w]))
                eng = (nc.gpsimd if (state["slot"] % 16) in POOL_SLOTS
                       else nc.vector)
                eng.tensor_mul(e3, e3, ebb)
                state["slot"] += 1
                return ew

            def emit_attnv(c, n0, n1, ew):
                w = n1 - n0
                nc.tensor.matmul(pattn[0][:, n0:n1],
                                 lhsT=v_sb[c][:, ha, 0:65],
                                 rhs=ew[:, 0:w],
                                 start=(c == 0), stop=(c == KT - 1))
                nc.tensor.matmul(pattn[1][:, n0:n1],
                                 lhsT=v_sb[c][:, hb, 0:65],
                                 rhs=ew[:, w:2 * w],
                                 start=(c == 0), stop=(c == KT - 1))

            def emit_attnv_pair(s0, s1):
                # both chunks of one kv-chunk: per head the two streams
                # run back-to-back under one weight set
                (c, n0, n1, ew0), (_, m0, m1, ew1) = s0, s1
                for hf, h in ((0, ha), (1, hb)):
                    nc.tensor.matmul(pattn[hf][:, n0:n1],
                                     lhsT=v_sb[c][:, h, 0:65],
                                     rhs=ew0[:, 0:512] if hf == 0
                                     else ew0[:, 512:1024],
                                     start=(c == 0), stop=(c == KT - 1))
                    nc.tensor.matmul(pattn[hf][:, m0:m1],
                                     lhsT=v_sb[c][:, h, 0:65],
                                     rhs=ew1[:, 0:512] if hf == 0
                                     else ew1[:, 512:1024],
                                     start=(c == 0), stop=(c == KT - 1))

            # pre-phase: run exp fronts ahead so ACT starts before v ready.
            # n0=0 only - the first fronts then gate on just q[:, 0:512],
            # which arrives ~15us before the full q tile.
            pre_ew = {}
            for c in range(pre):
                pre_ew[(c, 0)] = slot_front(c, 0, 512)
            if post_pre is not None:
                post_pre()

            slotq = []
            pend = []
            for c in range(KT):
                pend.extend(sched.get(c, ()))
                left = max(1, 2 * (KT - 2 - c))
                budget = (max(2, (len(pend) + left - 1) // left)
                          if c < KT - 2 else len(pend))
                for n0, n1 in _chunks(SL, 512):
                    ew = pre_ew.pop((c, n0), None)
                    if ew is None:
                        ew = slot_front(c, n0, n1)
                    slotq.append((c, n0, n1, ew))
                    if n0 == 512 and len(slotq) > 4:
                        s0 = slotq.pop(0)
                        s1 = slotq.pop(0)
                        emit_attnv_pair(s0, s1)
                    for th in pend[:budget]:
                        th()
                    del pend[:budget]
            for th in pend:
                th()
            while len(slotq) >= 2:
                emit_attnv_pair(slotq.pop(0), slotq.pop(0))
            for args in slotq:
                emit_attnv(*args)

            # ---- normalize: sau bf16 copy, denominator row to partitions
            # 0:2 via DMA, one reciprocal for both heads, partition
            # broadcast, 2x multiply.  qh0 rows bounce via audram.
            # Emitted as THUNKS released early in the NEXT iteration's
            # c-loop, so this ~6.5us DVE burst does not sit in front of
            # the next iteration's eb-muls in the DVE queue (sau copies
            # first - they gate the pattn pool rotation).
            den_t = smpool.tile([2, SL], BF, tag="dent", name=f"dn{qh}{hp}",
                                bufs=1)
            saus = [stpool.tile([65, SL], BF, tag="sau",
                                name=f"sa{qh}_{ha + hf}", bufs=2)
                    for hf in range(2)]
            row = qh * H + 2 * hp

            def sau_copy(hf):
                nc.vector.tensor_copy(saus[hf], pattn[hf])
                nc.sync.dma_start(out=den_t[hf:hf + 1, :],
                                  in_=saus[hf][64:65, :])

            def recip_chain():
                denf = smpool.tile([2, SL], F32, tag="denf",
                                   name=f"df{qh}{hp}", bufs=1)
                nc.vector.tensor_copy(denf, den_t)
                recipf = smpool.tile([2, SL], F32, tag="recipf",
                                     name=f"rf{qh}{hp}", bufs=1)
                nc.vector.reciprocal_approx_fast(recipf, denf)
                recipb = smpool.tile([2, SL], BF, tag="recipb",
                                     name=f"rb{qh}{hp}", bufs=1)
                nc.vector.tensor_copy(recipb, recipf)
                nc.sync.dma_start(out=rscr[row:row + 2, :], in_=recipb)

            def bc_mul(hf):
                bc = smpool.tile([DEPTH, SL], BF, tag="bc",
                                 name=f"bc{qh}_{ha + hf}", bufs=2)
                nc.sync.dma_start(
                    out=bc,
                    in_=rscr[row + hf:row + hf + 1, :].partition_broadcast(
                        DEPTH))
                if qh == 0 and not an_direct:
                    anh = smpool.tile([DEPTH, SL], BF, tag="anh",
                                      name=f"ah{ha + hf}", bufs=2)
                    nc.vector.tensor_mul(anh, saus[hf][0:64, :], bc)
                    nc.sync.dma_start(
                        out=audram[hp * P + hf * DEPTH:
                                   hp * P + (hf + 1) * DEPTH, :],
                        in_=anh)
                else:
                    an_sb = state["an"]
                    nc.vector.tensor_mul(
                        an_sb[hp][hf * DEPTH:(hf + 1) * DEPTH, q0:q0 + SL],
                        saus[hf][0:64, :], bc)

            if not fast_tail:
                return [lambda: sau_copy(0), lambda: sau_copy(1),
                        recip_chain, lambda: bc_mul(0), lambda: bc_mul(1)]

            # last iteration: fully on-chip normalize - PE gathers the
            # denominator row and broadcasts the reciprocal, no DRAM
            # round trips on the critical tail.
            ones = smpool.tile([P, DEPTH], BF, tag="ones", name="ones7",
                               bufs=1)
            nc.vector.memset(ones, 1.0)
            for hf in range(2):
                nc.vector.tensor_copy(saus[hf][64:65, :],
                                      pattn[hf][64:65, :])
                nc.vector.tensor_copy(saus[hf][0:64, :], pattn[hf][0:64, :])
            recips = []
            for hf in range(2):
                dps = plp.tile([P, 1024], F32, tag="pl", name=f"dps{hf}",
                               bufs=2)
                for n0, n1 in _chunks(SL, 512):
                    nc.tensor.matmul(dps[0:1, n0:n1],
                                     lhsT=ones[64:65, 0:1],
                                     rhs=saus[hf][64:65, n0:n1],
                                     start=True, stop=True)
                rcf = smpool.tile([2, SL], F32,
                                  tag="recipf" if hf == 0 else "denf",
                                  name=f"rcf7_{hf}", bufs=1)
                nc.vector.reciprocal_approx_fast(rcf[0:1, :], dps[0:1, :])
                rcb = smpool.tile([2, SL], BF,
                                  tag="recipb" if hf == 0 else "dent",
                                  name=f"rcb7_{hf}", bufs=1)
                nc.vector.tensor_copy(rcb[0:1, :], rcf[0:1, :])
                recips.append(rcb)
            an_sb = state["an"]
            for hf in range(2):
                bcp = plp.tile([P, 1024], F32, tag="pl", name=f"bcp{hf}",
                               bufs=2)
                for n0, n1 in _chunks(SL, 512):
                    nc.tensor.matmul(bcp[0:DEPTH, n0:n1],
                                     lhsT=ones[0:1, 0:DEPTH],
                                     rhs=recips[hf][0:1, n0:n1],
                                     start=True, stop=True)
                nc.vector.tensor_mul(
                    an_sb[hp][hf * DEPTH:(hf + 1) * DEPTH, q0:q0 + SL],
                    saus[hf][0:64, :], bcp[0:DEPTH, :])
            return []

        # iterations 0..2 run inside the full pool scope.  q projections
        # emit only the qh0 columns here; the qh1 halves run in it3 when
        # the PE has slack and ACT pacing is established.
        it0_thunks = k_thunks(0, 512, S) + v_thunks(2) + v_thunks(3)
        for c in range(4, KT):
            it0_thunks += v_thunks(c)
        it0_thunks += q_thunks(1, 0, SL) + k_thunks(1)
        nrm = emit_iter(0, 0, extra_thunks=it0_thunks, pre=6,
                        post_pre=post_pre0)
        nrm = emit_iter(0, 1, pre_thunks=nrm[0:2], pre=3,
                        extra_thunks=(nrm[2:] + q_thunks(2, 0, SL)
                                      + k_thunks(2) + q_thunks(3, 0, SL)))
        nrm = emit_iter(0, 2, pre_thunks=nrm[0:2], pre=3,
                        extra_thunks=nrm[2:] + k_thunks(3))

      # y/wk/wv closed; it3 runs the deferred qh1-half q projections
      # (x/wq still resident).
      q1thunks = (q_thunks(0, SL, S) + q_thunks(1, SL, S)
                  + q_thunks(2, SL, S) + q_thunks(3, SL, S))
      nrm = emit_iter(0, 3, pre_thunks=nrm[0:2], pre=3,
                      extra_thunks=nrm[2:] + q1thunks)
      state["nrm"] = nrm

    # x/wq closed: SBUF reused for eb(qh1), Wo, an, o staging.
    eb1pool = ctx.enter_context(tc.tile_pool(name="eb1pool", bufs=KT))
    wopool = ctx.enter_context(tc.tile_pool(name="wopool", bufs=WT))
    anpool = ctx.enter_context(tc.tile_pool(name="anpool", bufs=HT))
    opool = ctx.enter_context(tc.tile_pool(name="opool", bufs=5))
    eb1_sb = [eb1pool.tile([P, SL], BF, tag="eb1", name=f"eb1_{c}", bufs=KT)
              for c in range(KT)]
    wo_sb = [wopool.tile([P, D], BF, tag="wo", name=f"wo{t}", bufs=WT)
             for t in range(WT)]
    an_sb = [anpool.tile([P, S], BF, tag="an", name=f"an{t}", bufs=HT)
             for t in range(HT)]
    for c in range(KT):
        dq(eb1_sb[c], ebT[c * P:(c + 1) * P, SL:S])
    for t in range(WT):
        dq(wo_sb[t], woT[t * P:(t + 1) * P, :])
    # hp0-2's qh0 rows reload now; hp3's audram rows are written by it3's
    # normalize thunks (released inside it4), so its reload is an it4
    # thunk ordered after them.
    for t in range(HT - 1):
        nc.gpsimd.dma_start(out=an_sb[t][:, 0:SL],
                            in_=audram[t * P:(t + 1) * P, :])

    def reload_an3():
        nc.gpsimd.dma_start(out=an_sb[HT - 1][:, 0:SL],
                            in_=audram[(HT - 1) * P:HT * P, :])

    state["eb1"] = eb1_sb
    state["an"] = an_sb

    def emit_o_half(m, q0, cast_eng=None):
        # output rows m*128, query cols q0..q0+512 (one ~1.7us PE burst)
        ps = plp.tile([P, 1024], F32, tag="pl", name=f"pso{m}_{q0}", bufs=2)
        for t in range(WT):
            nc.tensor.matmul(ps[:, 0:512],
                             lhsT=wo_sb[t][:, m * P:(m + 1) * P],
                             rhs=an_sb[t][:, q0:q0 + 512],
                             start=(t == 0), stop=(t == WT - 1))
        osb = opool.tile([P, 512], F32, tag="osb", name=f"o{m}_{q0}",
                         bufs=5)
        if cast_eng is nc.scalar:
            nc.scalar.copy(osb, ps[:, 0:512])
        else:
            nc.vector.tensor_copy(osb, ps[:, 0:512])
        eng = (nc.sync, nc.gpsimd, nc.scalar)[(m + q0 // 512) % 3]
        eng.dma_start(out=outT[m * P:(m + 1) * P, q0:q0 + 512], in_=osb)

    nrm = state["nrm"]
    o0 = [lambda m=m, h=h: emit_o_half(m, h) for m in range(NT)
          for h in (0, 512)]
    # qh0 o-halves cast on ACT: it has structural gaps mid-stream while
    # the DVE queue is the muls' critical path
    o0s = [lambda m=m, h=h: emit_o_half(m, h, cast_eng=nc.scalar)
           for m in range(NT) for h in (0, 512)]
    nrm = emit_iter(1, 0, pre_thunks=nrm[0:2], pre=3,
                    extra_thunks=nrm[2:] + [reload_an3] + o0s[0:2])
    nrm = emit_iter(1, 1, pre_thunks=nrm[0:2], pre=3,
                    extra_thunks=nrm[2:] + o0s[2:9])
    nrm = emit_iter(1, 2, pre_thunks=nrm[0:2], pre=3,
                    extra_thunks=nrm[2:] + o0s[9:16])
    emit_iter(1, 3, pre_thunks=nrm[0:2], pre=3, extra_thunks=nrm[2:],
              fast_tail=True)
    for i, (m, h) in enumerate([(m, h) for m in range(NT)
                                for h in (SL, SL + 512)]):
        emit_o_half(m, h, cast_eng=nc.scalar if i % 2 else nc.vector)

    if DEBUG:
        for t in range(HT):
            nc.sync.dma_start(out=io["anD"][t * P:(t + 1) * P, :],
                              in_=an_sb[t])
            nc.sync.dma_start(out=io["qD"][t * P:(t + 1) * P, :],
                              in_=q_sb[t])
            nc.sync.dma_start(out=io["kD"][t * P:(t + 1) * P, :],
                              in_=k_sb[t])
        nc.sync.dma_start(out=io["rD"], in_=rscr)


def build_nc():
    nc = bacc.Bacc("TRN2", target_bir_lowering=False, debug=False)
    io = {
        "xT": nc.dram_tensor("xT", [D, S], BF, kind="ExternalInput").ap(),
        "yT": nc.dram_tensor("yT", [D, S], BF, kind="ExternalInput").ap(),
        "ebT": nc.dram_tensor("ebT", [S, S], BF, kind="ExternalInput").ap(),
        "wqT": nc.dram_tensor("wqT", [D, DH], BF, kind="ExternalInput").ap(),
        "wkT": nc.dram_tensor("wkT", [D, DH], BF, kind="ExternalInput").ap(),
        "wvT": nc.dram_tensor("wvT", [D, DH], BF, kind="ExternalInput").ap(),
        "woT": nc.dram_tensor("woT", [DH, D], BF, kind="ExternalInput").ap(),
        "outT": nc.dram_tensor("outT", [D, S], F32,
                               kind="ExternalOutput").ap(),
    }
    if DEBUG:
        io["anD"] = nc.dram_tensor("anD", [DH, S], BF,
                                   kind="ExternalOutput").ap()
        io["qD"] = nc.dram_tensor("qD", [DH, S], BF,
                                  kind="ExternalOutput").ap()
        io["kD"] = nc.dram_tensor("kD", [DH, S], BF,
                                  kind="ExternalOutput").ap()
        io["rD"] = nc.dram_tensor("rD", [2 * H, SL], BF,
                                  kind="ExternalOutput").ap()
    with tile.TileContext(nc) as tc:
        with ExitStack() as ctx:
            _attn_body(ctx, tc, io)
    nc.compile()
    return nc


_NC_CACHE = None


def kernel(x, y, bias, Wq, Wk, Wv, Wo):
    global _NC_CACHE, last_exec_time_ns, last_results
    x = np.asarray(x, np.float32)
    y = np.asarray(y, np.float32)
    bias = np.asarray(bias, np.float32)
    Wq, Wk, Wv, Wo = (np.asarray(w, np.float32) for w in (Wq, Wk, Wv, Wo))
    if _NC_CACHE is None:
        _NC_CACHE = build_nc()
    nc = _NC_CACHE

    bf = ml_dtypes.bfloat16
    ebT = np.ascontiguousarray(
        np.exp(bias[0, 0].astype(np.float32)).T).astype(bf)
    xT_all = [np.ascontiguousarray(x[b].T).astype(bf) for b in range(B)]
    yT_all = [np.ascontiguousarray(y[b].T).astype(bf) for b in range(B)]
    wqT = [np.ascontiguousarray(Wq[h * DH:(h + 1) * DH, :].T).astype(bf)
           for h in range(2)]
    wkT = [np.ascontiguousarray(Wk[h * DH:(h + 1) * DH, :].T).astype(bf)
           for h in range(2)]
    wvT = [np.ascontiguousarray(Wv[h * DH:(h + 1) * DH, :].T).astype(bf)
           for h in range(2)]
    woT = [np.ascontiguousarray(Wo.T[h * DH:(h + 1) * DH, :]).astype(bf)
           for h in range(2)]

    in_maps = []
    for core in range(NCORES):
        b, hh = divmod(core, 2)
        in_maps.append({
            "xT": xT_all[b], "yT": yT_all[b], "ebT": ebT,
            "wqT": wqT[hh], "wkT": wkT[hh], "wvT": wvT[hh],
            "woT": woT[hh],
        })

    res = run_bass_kernel_spmd(nc, in_maps, core_ids=list(range(NCORES)),
                               trace=TRACE)
    last_exec_time_ns = res.exec_time_ns
    last_results = res
    out = np.empty((B, S, D), np.float32)
    for b in range(B):
        tot = res.results[2 * b]["outT"] + res.results[2 * b + 1]["outT"]
        out[b] = tot.T
    return out
